# revision 8
# baseline (speedup 1.0000x reference)
"""Trainium2 Bass kernel for the quantum ConvLSTM reference.

Math reduction (validated to ~7e-3 rel vs the jax reference in numpy sim):
  * quantum_conv(patch) == T16[b] from thresholding the 4 pixels at 127;
    evaluated on-chip as a multilinear polynomial in the 4 bits.
  * qlayer(x, p) == [z1*z2*z3, z0*z1, z0*z1*z2, z0*z1*z2*z3] with
    z_w = cos(x_w + p_w); the LSTM scan becomes a small classical
    recurrence: theta = Wh@h + Wx*conv + beta; z = sin(theta + pi/2);
    gate products; sigmoid via (1+tanh(q/2))/2; c/h updates with the
    2c/2h scaling folded into constants.

Per-step pipeline (10 DVE ops per batch group, 2 groups interleaved
ABAB so every dependency sits at instruction distance >= 2 and the
vector engine's FIFO streams without semaphore stalls):
  M6   one multiply-cumsum of [h0..h3, conv_t, 1] x whx6 (96 els)
       -> full gate preactivation sums via guard-column differences
  DW   custom diff+wrap: wrap(CS[6m+6]-CS[6m]) into [-pi, pi]
  SIN  deg-7 odd sin (8 ALU stages, 4th coef via C3->Src1 latch)
  P1   strided multiply -> [q1, m12, b23] per gate unit
  P2   strided multiply -> [q2, q0, q3] per gate unit
  TS   1 + tanh(q/2) for sigmoid gates (deg-5, 0.5 folded into coefs)
  TU   tanh(q) for the u gate (deg-5)
  AB   paired multiply [u_f*s2, u_i*t_u]
  S2   s2' = 0.5*A + B (stock STT)
  HF   h' = u_o * tanh5(s2'/2), written into the next hE state block

Sharding: pure data parallel over batch (2048 -> 8 cores x 256 rows).
Each core: 128 partitions x 2 column-groups; seq scan of 512 steps fully
unrolled on-chip.
"""

import sys

import numpy as np

sys.path.insert(0, "/opt/trn_rl_repo")

N_CORES = 8
BATCH = 2048
SEQ = 512
BPC = BATCH // N_CORES          # 256 batch rows per core
GRP = BPC // 128                # 2 column groups of 128 partitions
PI = float(np.pi)

# Z-tile slot layout per gate unit (stride UBLK per unit a):
#   z0@0 z1@6 z2@12 z3@18   (sin output, stride 6)
#   q2@21 q0@22 q3@23       (P2 output, stride 1)
#   q1@24 m12@25 b23@26     (P1 output, stride 1)
#   th@28..31               (DW output)
# The tanh stage reads the contiguous window [21..25) = [q2,q0,q3,q1],
# i.e. wire order PERM = [2,0,3,1]. All per-wire state (u, tu, s2, A, B,
# h-slots in hE) is stored in this permuted order; whx's Wh columns and
# W_out are permuted host-side to match, so every AP is a plain
# positive-stride window.
UBLK = 32
PERM = [2, 0, 3, 1]

# STATE-tile slot layout per group:
#   s2@0..3  tu@4..7  u_f@8..11 u_i@12..15 u_o@16..19  A@20..23 B@24..27
SBLK = 32

# ---------------------------------------------------------------------------
# Host-side constants: T16 lookup table + multilinear coefficients.
# ---------------------------------------------------------------------------
_RY_ANGLES = np.random.RandomState(0).uniform(0.0, 2.0 * np.pi, size=(2, 4)).astype(np.float32)


def _build_t16() -> np.ndarray:
    s = np.zeros((16, 2, 2, 2, 2), np.complex64)
    for b in range(16):
        bits = [(b >> 3) & 1, (b >> 2) & 1, (b >> 1) & 1, b & 1]
        s[(b, *bits)] = 1.0

    def ry(state, th, w):
        a0 = np.take(state, 0, axis=1 + w)
        a1 = np.take(state, 1, axis=1 + w)
        c = np.complex64(np.cos(np.float32(th) / 2))
        sn = np.complex64(np.sin(np.float32(th) / 2))
        return np.stack([c * a0 - sn * a1, sn * a0 + c * a1], axis=1 + w)

    def cnot(state, ctl, tgt):
        s0 = np.take(state, 0, axis=1 + ctl)
        s1 = np.take(state, 1, axis=1 + ctl)
        t_ax = 1 + tgt if tgt < ctl else tgt
        s1 = np.flip(s1, axis=t_ax)
        return np.stack([s0, s1], axis=1 + ctl)

    for layer in range(2):
        for w in range(4):
            s = ry(s, _RY_ANGLES[layer, w], w)
        for w in range(3):
            s = cnot(s, w, w + 1)
    probs = np.abs(s) ** 2
    cols = []
    for w in range(4):
        other = tuple(a for a in range(1, 5) if a != 1 + w)
        cols.append(probs.sum(axis=other)[:, 1])
    return np.stack(cols, axis=1).mean(axis=1)  # (16,)


def _multilinear_coeffs(t16: np.ndarray) -> np.ndarray:
    """C[4][4] with T16[b] = sum_jk C[j,k]*u_j*v_k, u=[1,b0,b1,b0b1], v=[1,b2,b3,b2b3]."""
    m = np.zeros((16, 16))
    for b in range(16):
        b0, b1, b2, b3 = (b >> 3) & 1, (b >> 2) & 1, (b >> 1) & 1, b & 1
        u = [1, b0, b1, b0 * b1]
        v = [1, b2, b3, b2 * b3]
        for j in range(4):
            for k in range(4):
                m[b, j * 4 + k] = u[j] * v[k]
    return np.linalg.solve(m, t16.astype(np.float64)).reshape(4, 4)


_T16 = _build_t16()
_CML = _multilinear_coeffs(_T16)


def _fit_odd(f, hi, ncoef):
    """Near-minimax odd fit f(x) ~ x*p(x^2) on [-hi, hi]; p coeffs ascending."""
    n = 4000
    k = np.arange(n)
    x = np.cos(np.pi * (k + 0.5) / n) * hi
    y = x * x
    a = np.stack([x * y ** j for j in range(ncoef)], axis=1)
    tgt = f(x)
    c, *_ = np.linalg.lstsq(a, tgt, rcond=None)
    for _ in range(60):
        r = a @ c - tgt
        w = (np.abs(r) + 1e-12) ** 0.5
        c, *_ = np.linalg.lstsq(a * w[:, None], tgt * w, rcond=None)
    return c


_SIN7 = _fit_odd(np.sin, np.pi, 4)       # deg-7 odd sin on [-pi, pi], ~2.6e-4
_TG5 = _fit_odd(np.tanh, 1.0, 3)         # deg-5 tanh on [-1, 1], ~4e-4
_TC5 = _fit_odd(np.tanh, 2.0, 3)         # deg-5 tanh on [-2, 2] for tanh(c)
# tanh(q/2) coefs: c_k' = c_k / 2^(2k+1)
_TG5H = _TG5 * np.array([1 / 2 ** (2 * k + 1) for k in range(3)])
# tanh(s2/2) coefs for HFUSE (s2 = 2c)
_TC5H = _TC5 * np.array([1 / 2 ** (2 * k + 1) for k in range(3)])

_CACHE = {}


def _register_custom_ops():
    """Register fused DVE ops (idempotent). Shas are pinned by bootstrap:
    compile once with an empty pin, parse the actual sha from the error."""
    import re
    import concourse.dve_ops as dve_ops_mod
    from concourse.dve_ops import OPS, DveOp
    from concourse.dve_spec import (
        Spec, Src0, Src1, C0, C1, C2, Zero, One, Latch, scan, AluOp, sq,
    )

    have = {o.name for o in OPS}

    def make(name, spec):
        if name in have:
            return next(o for o in OPS if o.name == name)
        probe = DveOp(name, spec, subdim=False, uops_sha={})
        OPS.append(probe)
        dve_ops_mod._SUB_OPCODE_FOR_NAME[name] = (
            dve_ops_mod._CUSTOM_DVE_ROW_BASE + len(OPS) - 1)
        shas = {}
        for ver in ("v3", "v4"):
            try:
                probe.compile(ver)
            except ValueError as e:
                mm = re.search(r"(\b[0-9a-f]{16})\b", str(e))
                shas[ver] = mm.group(1)
        OPS.remove(probe)
        op = DveOp(name, spec, subdim=False, uops_sha=shas)
        OPS.append(op)
        return op

    # out = cumsum(in0 * in1) along the free stream (same as baseline)
    def _ref_mulscan(in0, in1, c0, c1, c2):
        p = (in0.astype(np.float32) * in1).reshape(in0.shape[0], -1)
        return np.cumsum(p, axis=1, dtype=np.float32).reshape(in0.shape)

    mulscan = make("MULSCAN_ANT", Spec(
        body=scan(AluOp.ADD, Src0 * Src1),
        reference=_ref_mulscan))

    # out = wrap(in0 - in1) into [-c1, c1] by one period c2
    d = Src0 - Src1
    dw = d + C2 * ((d < (Zero - C1)) - (C1 < d))
    def _ref_dw(in0, in1, c0, c1, c2):
        dd = (in0.astype(np.float32) - in1).astype(np.float32)
        return (dd + c2 * ((dd < -c1).astype(np.float32)
                           - (c1 < dd).astype(np.float32))).astype(np.float32)
    diffwrap = make("DIFFWRAP_ANT", Spec(body=dw, reference=_ref_dw))

    # out = x*(((c0*y + c1)*y + c2)*y + in1latch), y = x^2  (deg-7 odd)
    y7 = sq(Src0)
    p7 = (((C0 * y7 + C1) * y7 + C2) * y7 + Latch(Src1)) * Src0
    def _ref_p7(in0, in1, c0, c1, c2):
        yy = in0.astype(np.float32) ** 2
        c3 = in1.reshape(in1.shape[0], -1)[:, 0:1]
        c3 = c3.reshape((in1.shape[0],) + (1,) * (in0.ndim - 1))
        return ((((c0 * yy + c1) * yy + c2) * yy + c3) * in0).astype(np.float32)
    oddp7 = make("ODDP7_ANT", Spec(body=p7, reference=_ref_p7))

    # out = ((c0*y + c1)*y + c2)*x + 1, y = x^2  (deg-5 odd + 1)
    y5 = sq(Src0)
    t5p = ((C0 * y5 + C1) * y5 + C2) * Src0 + One
    def _ref_t5p(in0, in1, c0, c1, c2):
        yy = in0.astype(np.float32) ** 2
        return (((c0 * yy + c1) * yy + c2) * in0 + 1.0).astype(np.float32)
    tanh5p = make("TANH5P_ANT", Spec(body=t5p, reference=_ref_t5p))

    # out = ((c0*y + c1)*y + c2)*x, y = x^2  (deg-5 odd)
    t5 = ((C0 * y5 + C1) * y5 + C2) * Src0
    def _ref_t5(in0, in1, c0, c1, c2):
        yy = in0.astype(np.float32) ** 2
        return (((c0 * yy + c1) * yy + c2) * in0).astype(np.float32)
    tanh5 = make("TANH5_ANT", Spec(body=t5, reference=_ref_t5))

    # out = in1 * ((c0*y + c1)*y + c2)*in0, y = in0^2  (h update)
    hf = Src1 * (((C0 * y5 + C1) * y5 + C2) * Src0)
    def _ref_hf(in0, in1, c0, c1, c2):
        yy = in0.astype(np.float32) ** 2
        return (in1 * (((c0 * yy + c1) * yy + c2) * in0)).astype(np.float32)
    hfuse = make("HFUSE_ANT", Spec(body=hf, reference=_ref_hf))

    return mulscan, diffwrap, oddp7, tanh5p, tanh5, hfuse


def _build_program(debug=False):
    """Build + compile the (weights-independent) single-core SPMD Bass program."""
    import concourse.bass as bass
    import concourse.mybir as mybir
    import concourse.tile as tile
    from concourse import bacc

    F32 = mybir.dt.float32
    OP = mybir.AluOpType

    mulscan, diffwrap, oddp7, tanh5p, tanh5, hfuse = _register_custom_ops()

    nc = bacc.Bacc(None, target_bir_lowering=False)

    x_d = nc.dram_tensor("xs", [BPC, SEQ * 4], F32, kind="ExternalInput")
    wx_d = nc.dram_tensor("wx", [128, 96], F32, kind="ExternalInput")
    cp_d = nc.dram_tensor("cp", [128, 16], F32, kind="ExternalInput")
    y_d = nc.dram_tensor("y", [BPC, SEQ], F32, kind="ExternalOutput")
    if debug:
        dh_d = nc.dram_tensor("dbg_h", [128, GRP * (SEQ + 1) * 6], F32, kind="ExternalOutput")

    sc7 = [float(v) for v in _SIN7]
    tgh = [float(v) for v in _TG5H]
    tg = [float(v) for v in _TG5]
    tch = [float(v) for v in _TC5H]

    with tile.TileContext(nc) as tc:
        with (
            tc.tile_pool(name="big", bufs=1) as big,
            tc.tile_pool(name="ph1", bufs=1) as ph1,
        ):
            # ---------------- load ----------------
            xsb = big.tile([128, GRP * SEQ * 4], F32, tag="X")       # (g, t, k)
            nc.sync.dma_start(
                xsb[:].rearrange("p (g n) -> p g n", g=GRP),
                x_d.rearrange("(g p) n -> p g n", p=128),
            )
            wxsb = big.tile([128, 96], F32, tag="WX")                # (a, w, k6)
            nc.sync.dma_start(wxsb[:], wx_d[:])
            cpsb = big.tile([128, 16], F32, tag="CP")
            nc.sync.dma_start(cpsb[:], cp_d[:])

            # ---------------- phase 1: bits -> conv ----------------
            bsb = big.tile([128, GRP * SEQ * 4], F32, tag="B")
            nc.vector.tensor_scalar(out=bsb[:], in0=xsb[:], scalar1=127.0,
                                    scalar2=None, op0=OP.is_gt)
            bv = bsb[:].rearrange("p (g t k) -> p g t k", g=GRP, k=4)
            bk = [bv[:, :, :, k] for k in range(4)]                  # each (128, g, t)

            q01 = ph1.tile([128, GRP * SEQ], F32, tag="q01")
            q23 = ph1.tile([128, GRP * SEQ], F32, tag="q23")
            gt = lambda tl: tl[:].rearrange("p (g t) -> p g t", g=GRP)
            nc.vector.tensor_tensor(out=gt(q01), in0=bk[0], in1=bk[1], op=OP.mult)
            nc.vector.tensor_tensor(out=gt(q23), in0=bk[2], in1=bk[3], op=OP.mult)
            rs = []
            for j in range(4):
                r = ph1.tile([128, GRP * SEQ], F32, tag=f"r{j}")
                nc.vector.tensor_scalar(out=gt(r), in0=bk[2],
                                        scalar1=float(_CML[j, 1]),
                                        scalar2=float(_CML[j, 0]),
                                        op0=OP.mult, op1=OP.add)
                nc.vector.affine_then_add(out=gt(r), in0=bk[3], in1=gt(r),
                                          scale=float(_CML[j, 2]), bias=0.0)
                nc.vector.affine_then_add(out=gt(r), in0=gt(q23), in1=gt(r),
                                          scale=float(_CML[j, 3]), bias=0.0)
                rs.append(r)
            m = ph1.tile([128, GRP * SEQ], F32, tag="m")
            vcv = big.tile([128, GRP * SEQ], F32, tag="V")           # conv (g, t)
            nc.vector.tensor_tensor(out=gt(m), in0=bk[0], in1=gt(rs[1]), op=OP.mult)
            nc.vector.tensor_tensor(out=gt(vcv), in0=gt(rs[0]), in1=gt(m), op=OP.add)
            nc.vector.tensor_tensor(out=gt(m), in0=bk[1], in1=gt(rs[2]), op=OP.mult)
            nc.vector.tensor_tensor(out=gt(vcv), in0=gt(vcv), in1=gt(m), op=OP.add)
            nc.vector.tensor_tensor(out=gt(m), in0=gt(q01), in1=gt(rs[3]), op=OP.mult)
            nc.vector.tensor_tensor(out=gt(vcv), in0=gt(vcv), in1=gt(m), op=OP.add)

            # ---------------- phase 1b: populate hE state buffers ----------
            # hE_g block t (cols t*6 .. t*6+5) = [h0..h3, conv_t, 1.0]
            hE = [big.tile([128, (SEQ + 1) * 6], F32, tag=f"HE{g}", name=f"HE{g}") for g in range(GRP)]
            CS = [big.tile([128, 97], F32, tag=f"CS{g}", name=f"CS{g}") for g in range(GRP)]
            Z = [big.tile([128, 4 * UBLK], F32, tag=f"Z{g}", name=f"Z{g}") for g in range(GRP)]
            STA = [big.tile([128, 8], F32, tag=f"STA{g}", name=f"STA{g}") for g in range(GRP)]
            STB = [big.tile([128, 12], F32, tag=f"STB{g}", name=f"STB{g}") for g in range(GRP)]
            STC = [big.tile([128, 8], F32, tag=f"STC{g}", name=f"STC{g}") for g in range(GRP)]

            for g in range(GRP):
                hv = hE[g][:].rearrange("p (t k) -> p t k", k=6)
                # conv slots: hE[t*6+4] = vcv[g, t]
                nc.vector.tensor_scalar(
                    out=hv[:, 0:SEQ, 4], in0=vcv[:].rearrange(
                        "p (g t) -> p g t", g=GRP)[:, g, :],
                    scalar1=1.0, scalar2=None, op0=OP.mult)
                nc.vector.memset(hv[:, 0:SEQ, 5], 1.0)
                nc.vector.memset(hv[:, 0, 0:4], 0.0)
                nc.vector.memset(STA[g][:, 0:4], 0.0)      # s2 = 0
                nc.vector.memset(CS[g][:, 0:1], 0.0)       # guard column

            # ---------------- phase 2: the scan ----------------
            zv = [Z[g][:].rearrange("p (a u) -> p a u", u=UBLK) for g in range(GRP)]
            GPPROBE = True
            if GPPROBE:
                gpd = [big.tile([128, 8], F32, tag=f"GPD{g}", name=f"GPD{g}") for g in range(GRP)]
            for t in range(SEQ):
                if GPPROBE:
                    for g in range(GRP):
                        nc.gpsimd.tensor_tensor(
                            out=gpd[g][:], in0=vcv[:, 0:8], in1=vcv[:, 8:16],
                            op=OP.mult)
                        nc.gpsimd.tensor_tensor(
                            out=gpd[g][:], in0=vcv[:, 16:24], in1=vcv[:, 24:32],
                            op=OP.add)
                # emit in ABAB (group-interleaved) order per stage so every
                # dependency sits at instruction distance >= 2
                for g in range(GRP):  # M6
                    hblk = hE[g][:, t * 6:(t + 1) * 6].unsqueeze(1)
                    nc.vector._custom_dve(
                        mulscan,
                        out=CS[g][:, 1:97].rearrange("p (m k) -> p m k", k=6),
                        in0=hblk.broadcast_to((128, 16, 6)),
                        in1=wxsb[:].rearrange("p (m k) -> p m k", k=6))
                for g in range(GRP):  # DW: th = wrap(ends - starts) -> th slots
                    ends = CS[g][:, 1:97].rearrange("p (m k) -> p m k", k=6)[:, :, 5]
                    starts = CS[g][:, 0:96].rearrange("p (m k) -> p m k", k=6)[:, :, 0]
                    nc.vector._custom_dve(
                        diffwrap,
                        out=zv[g][:, :, 28:32],
                        in0=ends, in1=starts,
                        s0=0.0, s1=PI, imm2=2 * PI)
                for g in range(GRP):  # SIN: z_w at slots 0,6,12,18
                    nc.vector._custom_dve(
                        oddp7,
                        out=zv[g][:, :, 0:24].rearrange("p a (w r) -> p a w r", r=6)[:, :, :, 0],
                        in0=zv[g][:, :, 28:32],
                        in1=cpsb[:, 0:1],
                        s0=sc7[3], s1=sc7[2], imm2=sc7[1])
                for g in range(GRP):  # P1: [q1,m12,b23] = z[0,1,2]*z[1,2,3]
                    nc.vector.tensor_tensor(
                        out=zv[g][:, :, 24:27],
                        in0=zv[g][:, :, 0:18].rearrange("p a (w r) -> p a w r", r=6)[:, :, :, 0],
                        in1=zv[g][:, :, 6:24].rearrange("p a (w r) -> p a w r", r=6)[:, :, :, 0],
                        op=OP.mult)
                for g in range(GRP):  # P2: [q2,q0,q3] = [z2,z3,q1]*[q1,m12,b23]
                    nc.vector.tensor_tensor(
                        out=zv[g][:, :, 21:24],
                        in0=zv[g][:, :, 12:30].rearrange("p a (w r) -> p a w r", r=6)[:, :, :, 0],
                        in1=zv[g][:, :, 24:27],
                        op=OP.mult)
                for g in range(GRP):  # TS: u_a = 1 + tanh(q/2), a in {f,i,o}
                    # in0 = contiguous [q2,q0,q3,q1] window; PERM order
                    nc.vector._custom_dve(
                        tanh5p,
                        out=STB[g][:, 0:12].rearrange("p (a w) -> p a w", w=4),
                        in0=zv[g][:, 0:3, 21:25],
                        s0=tgh[2], s1=tgh[1], imm2=tgh[0])
                for g in range(GRP):  # TU: t_u = tanh(q), a = u
                    nc.vector._custom_dve(
                        tanh5,
                        out=STA[g][:, 4:8],
                        in0=zv[g][:, 3, 21:25],
                        s0=tg[2], s1=tg[1], imm2=tg[0])
                for g in range(GRP):  # AB: [A,B] = [u_f,u_i] * [s2,tu]
                    nc.vector.tensor_tensor(
                        out=STC[g][:].rearrange("p (j w) -> p j w", w=4),
                        in0=STB[g][:, 0:8].rearrange("p (j w) -> p j w", w=4),
                        in1=STA[g][:].rearrange("p (j w) -> p j w", w=4),
                        op=OP.mult)
                for g in range(GRP):  # S2: s2' = 0.5*A + B
                    nc.vector.scalar_tensor_tensor(
                        out=STA[g][:, 0:4], in0=STC[g][:, 0:4], scalar=0.5,
                        in1=STC[g][:, 4:8], op0=OP.mult, op1=OP.add)
                for g in range(GRP):  # HF: h' = u_o * tanh5(s2'/2) -> hE[t+1]
                    nc.vector._custom_dve(
                        hfuse,
                        out=hE[g][:, (t + 1) * 6:(t + 1) * 6 + 4],
                        in0=STA[g][:, 0:4],
                        in1=STB[g][:, 8:12],
                        s0=tch[2], s1=tch[1], imm2=tch[0])

            # ---------------- phase 3: y = h @ (W_out/2) + b_out ----------
            yt = big.tile([128, GRP * SEQ * 4], F32, tag="YT")
            for g in range(GRP):
                hsv = (hE[g][:, 6:].rearrange("p (t k) -> p t k", k=6)[:, :, 0:4])
                wo = (cpsb[:, 4:8].unsqueeze(1).broadcast_to((128, SEQ, 4)))
                nc.vector.tensor_tensor(
                    out=yt[:].rearrange("p (g t w) -> p g t w", g=GRP, t=SEQ)[:, g],
                    in0=hsv, in1=wo, op=OP.mult)
            yy = big.tile([128, GRP * SEQ], F32, tag="Y")
            nc.vector.tensor_reduce(
                out=yy[:], in_=yt[:].rearrange("p (m w) -> p m w", w=4),
                axis=mybir.AxisListType.X, op=OP.add)
            nc.vector.tensor_scalar(out=yy[:], in0=yy[:],
                                    scalar1=cpsb[:, 8:9], scalar2=None,
                                    op0=OP.add)
            nc.sync.dma_start(
                y_d.rearrange("(g p) t -> p g t", p=128),
                yy[:].rearrange("p (g t) -> p g t", g=GRP),
            )
            if debug:
                for g in range(GRP):
                    nc.sync.dma_start(
                        dh_d[:].rearrange("p (g n) -> p g n", g=GRP)[:, g],
                        hE[g][:])

    nc.compile()
    return nc


def _pack_consts(W_f, b_f, W_i, b_i, W_u, b_u, W_o, b_o,
                 rx_f, rx_i, rx_u, rx_o, W_out, b_out):
    """wx[128,96] (m=(a,w), k6=[0.5*Wh permuted, Wx, beta']) and cp[128,16].

    hE h-slots hold h in PERM wire order, so whx's Wh columns are permuted
    to match: whx[m, j] = 0.5*W[a][w_m, 1 + PERM[j]]."""
    Ws = [W_f, W_i, W_o, W_u]          # gate order f,i,o,u
    bs = [b_f, b_i, b_o, b_u]
    rxs = [rx_f, rx_i, rx_o, rx_u]
    whx = np.zeros((4, 4, 6), np.float32)
    for a in range(4):
        Wa = np.asarray(Ws[a], np.float32)
        whx[a, :, 0:4] = 0.5 * Wa[:, 1:5][:, PERM]
        whx[a, :, 4] = Wa[:, 0]
        beta = (np.asarray(bs[a], np.float32)
                + np.asarray(rxs[a], np.float32) + np.float32(np.pi / 2))
        whx[a, :, 5] = (beta + np.pi) % (2 * np.pi) - np.pi
    wx = np.tile(whx.reshape(1, 96), (128, 1)).astype(np.float32)

    cprow = np.zeros(16, np.float32)
    cprow[0] = float(_SIN7[0])                        # sin c3 latch const
    cprow[4:8] = 0.5 * np.asarray(W_out, np.float32)[0][PERM]
    cprow[8] = float(np.asarray(b_out, np.float32)[0])
    cp = np.tile(cprow[None], (128, 1)).astype(np.float32)

    # range check: theta must stay within [-3pi, 3pi] for the single wrap
    whabs = np.abs(whx[:, :, 0:4]).sum(axis=2) * 2.0
    bound = (np.abs(whx[:, :, 5]) + np.abs(whx[:, :, 4]) + whabs).max()
    assert bound < 3 * PI - 0.2, f"theta range {bound} too large for single wrap"
    return wx, cp


def kernel(**inputs):
    from concourse.bass_utils import run_bass_kernel_spmd

    x = np.ascontiguousarray(np.asarray(inputs["x"], np.float32)).reshape(BATCH, SEQ, 4)
    wx, cp = _pack_consts(**{k: v for k, v in inputs.items() if k != "x"})

    if "nc" not in _CACHE:
        _CACHE["nc"] = _build_program()
    nc = _CACHE["nc"]

    in_maps = []
    for cid in range(N_CORES):
        xs = np.ascontiguousarray(
            x[cid * BPC:(cid + 1) * BPC].reshape(BPC, SEQ * 4))
        in_maps.append({"xs": xs, "wx": wx, "cp": cp})

    res = run_bass_kernel_spmd(nc, in_maps, core_ids=list(range(N_CORES)))
    ys = [res.results[cid]["y"] for cid in range(N_CORES)]  # each (BPC, SEQ)
    full = np.concatenate(ys, axis=0)                       # (BATCH, SEQ)
    return np.ascontiguousarray(full.T)[:, :, None].astype(np.float32)


# revision 10
# speedup vs baseline: 1.0192x; 1.0192x over previous
"""Trainium2 Bass kernel for the quantum ConvLSTM reference.

Math reduction (validated to ~7e-3 rel vs the jax reference in numpy sim):
  * quantum_conv(patch) == T16[b] from thresholding the 4 pixels at 127;
    evaluated on-chip as a multilinear polynomial in the 4 bits.
  * qlayer(x, p) == [z1*z2*z3, z0*z1, z0*z1*z2, z0*z1*z2*z3] with
    z_w = cos(x_w + p_w); the LSTM scan becomes a small classical
    recurrence: theta = Wh@h + Wx*conv + beta; z = sin(theta + pi/2);
    gate products; sigmoid via (1+tanh(q/2))/2; c/h updates with the
    2c/2h scaling folded into constants.

Per-step pipeline (10 DVE ops per batch group, 2 groups interleaved
ABAB so every dependency sits at instruction distance >= 2 and the
vector engine's FIFO streams without semaphore stalls):
  M6   one multiply-cumsum of [h0..h3, conv_t, 1] x whx6 (96 els)
       -> full gate preactivation sums via guard-column differences
  DW   custom diff+wrap: wrap(CS[6m+6]-CS[6m]) into [-pi, pi]
  SIN  deg-7 odd sin (8 ALU stages, 4th coef via C3->Src1 latch)
  P1   strided multiply -> [q1, m12, b23] per gate unit
  P2   strided multiply -> [q2, q0, q3] per gate unit
  TS   1 + tanh(q/2) for sigmoid gates (deg-5, 0.5 folded into coefs)
  TU   tanh(q) for the u gate (deg-5)
  AB   paired multiply [u_f*s2, u_i*t_u]
  S2   s2' = 0.5*A + B (stock STT)
  HF   h' = u_o * tanh5(s2'/2), written into the next hE state block

Sharding: pure data parallel over batch (2048 -> 8 cores x 256 rows).
Each core: 128 partitions x 2 column-groups; seq scan of 512 steps fully
unrolled on-chip.
"""

import sys

import numpy as np

sys.path.insert(0, "/opt/trn_rl_repo")

N_CORES = 8
BATCH = 2048
SEQ = 512
BPC = BATCH // N_CORES          # 256 batch rows per core
GRP = BPC // 128                # 2 column groups of 128 partitions
PI = float(np.pi)

# Z-tile slot layout per gate unit (stride UBLK per unit a):
#   z0@0 z1@6 z2@12 z3@18   (sin output, stride 6)
#   q2@21 q0@22 q3@23       (P2 output, stride 1)
#   q1@24 m12@25 b23@26     (P1 output, stride 1)
#   th@28..31               (DW output)
# The tanh stage reads the contiguous window [21..25) = [q2,q0,q3,q1],
# i.e. wire order PERM = [2,0,3,1]. All per-wire state (u, tu, s2, A, B,
# h-slots in hE) is stored in this permuted order; whx's Wh columns and
# W_out are permuted host-side to match, so every AP is a plain
# positive-stride window.
UBLK = 32
PERM = [2, 0, 3, 1]

# STATE-tile slot layout per group:
#   s2@0..3  tu@4..7  u_f@8..11 u_i@12..15 u_o@16..19  A@20..23 B@24..27
SBLK = 32

# ---------------------------------------------------------------------------
# Host-side constants: T16 lookup table + multilinear coefficients.
# ---------------------------------------------------------------------------
_RY_ANGLES = np.random.RandomState(0).uniform(0.0, 2.0 * np.pi, size=(2, 4)).astype(np.float32)


def _build_t16() -> np.ndarray:
    s = np.zeros((16, 2, 2, 2, 2), np.complex64)
    for b in range(16):
        bits = [(b >> 3) & 1, (b >> 2) & 1, (b >> 1) & 1, b & 1]
        s[(b, *bits)] = 1.0

    def ry(state, th, w):
        a0 = np.take(state, 0, axis=1 + w)
        a1 = np.take(state, 1, axis=1 + w)
        c = np.complex64(np.cos(np.float32(th) / 2))
        sn = np.complex64(np.sin(np.float32(th) / 2))
        return np.stack([c * a0 - sn * a1, sn * a0 + c * a1], axis=1 + w)

    def cnot(state, ctl, tgt):
        s0 = np.take(state, 0, axis=1 + ctl)
        s1 = np.take(state, 1, axis=1 + ctl)
        t_ax = 1 + tgt if tgt < ctl else tgt
        s1 = np.flip(s1, axis=t_ax)
        return np.stack([s0, s1], axis=1 + ctl)

    for layer in range(2):
        for w in range(4):
            s = ry(s, _RY_ANGLES[layer, w], w)
        for w in range(3):
            s = cnot(s, w, w + 1)
    probs = np.abs(s) ** 2
    cols = []
    for w in range(4):
        other = tuple(a for a in range(1, 5) if a != 1 + w)
        cols.append(probs.sum(axis=other)[:, 1])
    return np.stack(cols, axis=1).mean(axis=1)  # (16,)


def _multilinear_coeffs(t16: np.ndarray) -> np.ndarray:
    """C[4][4] with T16[b] = sum_jk C[j,k]*u_j*v_k, u=[1,b0,b1,b0b1], v=[1,b2,b3,b2b3]."""
    m = np.zeros((16, 16))
    for b in range(16):
        b0, b1, b2, b3 = (b >> 3) & 1, (b >> 2) & 1, (b >> 1) & 1, b & 1
        u = [1, b0, b1, b0 * b1]
        v = [1, b2, b3, b2 * b3]
        for j in range(4):
            for k in range(4):
                m[b, j * 4 + k] = u[j] * v[k]
    return np.linalg.solve(m, t16.astype(np.float64)).reshape(4, 4)


_T16 = _build_t16()
_CML = _multilinear_coeffs(_T16)


def _fit_odd(f, hi, ncoef):
    """Near-minimax odd fit f(x) ~ x*p(x^2) on [-hi, hi]; p coeffs ascending."""
    n = 4000
    k = np.arange(n)
    x = np.cos(np.pi * (k + 0.5) / n) * hi
    y = x * x
    a = np.stack([x * y ** j for j in range(ncoef)], axis=1)
    tgt = f(x)
    c, *_ = np.linalg.lstsq(a, tgt, rcond=None)
    for _ in range(60):
        r = a @ c - tgt
        w = (np.abs(r) + 1e-12) ** 0.5
        c, *_ = np.linalg.lstsq(a * w[:, None], tgt * w, rcond=None)
    return c


_SIN7 = _fit_odd(np.sin, np.pi, 4)       # deg-7 odd sin on [-pi, pi], ~2.6e-4
_TG5 = _fit_odd(np.tanh, 1.0, 3)         # deg-5 tanh on [-1, 1], ~4e-4
_TC5 = _fit_odd(np.tanh, 2.0, 3)         # deg-5 tanh on [-2, 2] for tanh(c)
# tanh(q/2) coefs: c_k' = c_k / 2^(2k+1)
_TG5H = _TG5 * np.array([1 / 2 ** (2 * k + 1) for k in range(3)])
# tanh(s2/2) coefs for HFUSE (s2 = 2c)
_TC5H = _TC5 * np.array([1 / 2 ** (2 * k + 1) for k in range(3)])

_CACHE = {}


def _register_custom_ops():
    """Register fused DVE ops (idempotent). Shas are pinned by bootstrap:
    compile once with an empty pin, parse the actual sha from the error."""
    import re
    import concourse.dve_ops as dve_ops_mod
    from concourse.dve_ops import OPS, DveOp
    from concourse.dve_spec import (
        Spec, Src0, Src1, C0, C1, C2, Zero, One, Latch, scan, AluOp, sq,
    )

    have = {o.name for o in OPS}

    def make(name, spec):
        if name in have:
            return next(o for o in OPS if o.name == name)
        probe = DveOp(name, spec, subdim=False, uops_sha={})
        OPS.append(probe)
        dve_ops_mod._SUB_OPCODE_FOR_NAME[name] = (
            dve_ops_mod._CUSTOM_DVE_ROW_BASE + len(OPS) - 1)
        shas = {}
        for ver in ("v3", "v4"):
            try:
                probe.compile(ver)
            except ValueError as e:
                mm = re.search(r"(\b[0-9a-f]{16})\b", str(e))
                shas[ver] = mm.group(1)
        OPS.remove(probe)
        op = DveOp(name, spec, subdim=False, uops_sha=shas)
        OPS.append(op)
        return op

    # out = cumsum(in0 * in1) along the free stream (same as baseline)
    def _ref_mulscan(in0, in1, c0, c1, c2):
        p = (in0.astype(np.float32) * in1).reshape(in0.shape[0], -1)
        return np.cumsum(p, axis=1, dtype=np.float32).reshape(in0.shape)

    mulscan = make("MULSCAN_ANT", Spec(
        body=scan(AluOp.ADD, Src0 * Src1),
        reference=_ref_mulscan))

    # out = wrap(in0 - in1) into [-c1, c1] by one period c2
    d = Src0 - Src1
    dw = d + C2 * ((d < (Zero - C1)) - (C1 < d))
    def _ref_dw(in0, in1, c0, c1, c2):
        dd = (in0.astype(np.float32) - in1).astype(np.float32)
        return (dd + c2 * ((dd < -c1).astype(np.float32)
                           - (c1 < dd).astype(np.float32))).astype(np.float32)
    diffwrap = make("DIFFWRAP_ANT", Spec(body=dw, reference=_ref_dw))

    # out = x*(((c0*y + c1)*y + c2)*y + in1latch), y = x^2  (deg-7 odd)
    y7 = sq(Src0)
    p7 = (((C0 * y7 + C1) * y7 + C2) * y7 + Latch(Src1)) * Src0
    def _ref_p7(in0, in1, c0, c1, c2):
        yy = in0.astype(np.float32) ** 2
        c3 = in1.reshape(in1.shape[0], -1)[:, 0:1]
        c3 = c3.reshape((in1.shape[0],) + (1,) * (in0.ndim - 1))
        return ((((c0 * yy + c1) * yy + c2) * yy + c3) * in0).astype(np.float32)
    oddp7 = make("ODDP7_ANT", Spec(body=p7, reference=_ref_p7))

    # out = ((c0*y + c1)*y + c2)*x + 1, y = x^2  (deg-5 odd + 1)
    y5 = sq(Src0)
    t5p = ((C0 * y5 + C1) * y5 + C2) * Src0 + One
    def _ref_t5p(in0, in1, c0, c1, c2):
        yy = in0.astype(np.float32) ** 2
        return (((c0 * yy + c1) * yy + c2) * in0 + 1.0).astype(np.float32)
    tanh5p = make("TANH5P_ANT", Spec(body=t5p, reference=_ref_t5p))

    # out = ((c0*y + c1)*y + c2)*x, y = x^2  (deg-5 odd)
    t5 = ((C0 * y5 + C1) * y5 + C2) * Src0
    def _ref_t5(in0, in1, c0, c1, c2):
        yy = in0.astype(np.float32) ** 2
        return (((c0 * yy + c1) * yy + c2) * in0).astype(np.float32)
    tanh5 = make("TANH5_ANT", Spec(body=t5, reference=_ref_t5))

    # out = in1 * ((c0*y + c1)*y + c2)*in0, y = in0^2  (h update)
    hf = Src1 * (((C0 * y5 + C1) * y5 + C2) * Src0)
    def _ref_hf(in0, in1, c0, c1, c2):
        yy = in0.astype(np.float32) ** 2
        return (in1 * (((c0 * yy + c1) * yy + c2) * in0)).astype(np.float32)
    hfuse = make("HFUSE_ANT", Spec(body=hf, reference=_ref_hf))

    return mulscan, diffwrap, oddp7, tanh5p, tanh5, hfuse


def _build_program(debug=False):
    """Build + compile the (weights-independent) single-core SPMD Bass program."""
    import concourse.bass as bass
    import concourse.mybir as mybir
    import concourse.tile as tile
    from concourse import bacc

    F32 = mybir.dt.float32
    OP = mybir.AluOpType

    mulscan, diffwrap, oddp7, tanh5p, tanh5, hfuse = _register_custom_ops()

    nc = bacc.Bacc(None, target_bir_lowering=False)

    x_d = nc.dram_tensor("xs", [BPC, SEQ * 4], F32, kind="ExternalInput")
    wx_d = nc.dram_tensor("wx", [128, 96], F32, kind="ExternalInput")
    cp_d = nc.dram_tensor("cp", [128, 16], F32, kind="ExternalInput")
    y_d = nc.dram_tensor("y", [BPC, SEQ], F32, kind="ExternalOutput")
    if debug:
        dh_d = nc.dram_tensor("dbg_h", [128, GRP * (SEQ + 1) * 6], F32, kind="ExternalOutput")

    sc7 = [float(v) for v in _SIN7]
    tgh = [float(v) for v in _TG5H]
    tg = [float(v) for v in _TG5]
    tch = [float(v) for v in _TC5H]

    with tile.TileContext(nc) as tc:
        with (
            tc.tile_pool(name="big", bufs=1) as big,
            tc.tile_pool(name="ph1", bufs=1) as ph1,
        ):
            # ---------------- load ----------------
            xsb = big.tile([128, GRP * SEQ * 4], F32, tag="X")       # (g, t, k)
            nc.sync.dma_start(
                xsb[:].rearrange("p (g n) -> p g n", g=GRP),
                x_d.rearrange("(g p) n -> p g n", p=128),
            )
            wxsb = big.tile([128, 96], F32, tag="WX")                # (a, w, k6)
            nc.sync.dma_start(wxsb[:], wx_d[:])
            cpsb = big.tile([128, 16], F32, tag="CP")
            nc.sync.dma_start(cpsb[:], cp_d[:])

            # ---------------- phase 1: bits -> conv ----------------
            bsb = big.tile([128, GRP * SEQ * 4], F32, tag="B")
            nc.vector.tensor_scalar(out=bsb[:], in0=xsb[:], scalar1=127.0,
                                    scalar2=None, op0=OP.is_gt)
            bv = bsb[:].rearrange("p (g t k) -> p g t k", g=GRP, k=4)
            bk = [bv[:, :, :, k] for k in range(4)]                  # each (128, g, t)

            q01 = ph1.tile([128, GRP * SEQ], F32, tag="q01")
            q23 = ph1.tile([128, GRP * SEQ], F32, tag="q23")
            gt = lambda tl: tl[:].rearrange("p (g t) -> p g t", g=GRP)
            nc.vector.tensor_tensor(out=gt(q01), in0=bk[0], in1=bk[1], op=OP.mult)
            nc.vector.tensor_tensor(out=gt(q23), in0=bk[2], in1=bk[3], op=OP.mult)
            rs = []
            for j in range(4):
                r = ph1.tile([128, GRP * SEQ], F32, tag=f"r{j}")
                nc.vector.tensor_scalar(out=gt(r), in0=bk[2],
                                        scalar1=float(_CML[j, 1]),
                                        scalar2=float(_CML[j, 0]),
                                        op0=OP.mult, op1=OP.add)
                nc.vector.affine_then_add(out=gt(r), in0=bk[3], in1=gt(r),
                                          scale=float(_CML[j, 2]), bias=0.0)
                nc.vector.affine_then_add(out=gt(r), in0=gt(q23), in1=gt(r),
                                          scale=float(_CML[j, 3]), bias=0.0)
                rs.append(r)
            m = ph1.tile([128, GRP * SEQ], F32, tag="m")
            vcv = big.tile([128, GRP * SEQ], F32, tag="V")           # conv (g, t)
            nc.vector.tensor_tensor(out=gt(m), in0=bk[0], in1=gt(rs[1]), op=OP.mult)
            nc.vector.tensor_tensor(out=gt(vcv), in0=gt(rs[0]), in1=gt(m), op=OP.add)
            nc.vector.tensor_tensor(out=gt(m), in0=bk[1], in1=gt(rs[2]), op=OP.mult)
            nc.vector.tensor_tensor(out=gt(vcv), in0=gt(vcv), in1=gt(m), op=OP.add)
            nc.vector.tensor_tensor(out=gt(m), in0=gt(q01), in1=gt(rs[3]), op=OP.mult)
            nc.vector.tensor_tensor(out=gt(vcv), in0=gt(vcv), in1=gt(m), op=OP.add)

            # ---------------- phase 1b: populate hE state buffers ----------
            # hE_g block t (cols t*6 .. t*6+5) = [h0..h3, conv_t, 1.0]
            hE = [big.tile([128, (SEQ + 1) * 6], F32, tag=f"HE{g}", name=f"HE{g}") for g in range(GRP)]
            CS = [big.tile([128, 97], F32, tag=f"CS{g}", name=f"CS{g}") for g in range(GRP)]
            Z = [big.tile([128, 4 * UBLK], F32, tag=f"Z{g}", name=f"Z{g}") for g in range(GRP)]
            STA = [big.tile([128, 8], F32, tag=f"STA{g}", name=f"STA{g}") for g in range(GRP)]
            STB = [big.tile([128, 12], F32, tag=f"STB{g}", name=f"STB{g}") for g in range(GRP)]
            STC = [big.tile([128, 8], F32, tag=f"STC{g}", name=f"STC{g}") for g in range(GRP)]

            for g in range(GRP):
                hv = hE[g][:].rearrange("p (t k) -> p t k", k=6)
                # conv slots: hE[t*6+4] = vcv[g, t]
                nc.vector.tensor_scalar(
                    out=hv[:, 0:SEQ, 4], in0=vcv[:].rearrange(
                        "p (g t) -> p g t", g=GRP)[:, g, :],
                    scalar1=1.0, scalar2=None, op0=OP.mult)
                nc.vector.memset(hv[:, 0:SEQ, 5], 1.0)
                nc.vector.memset(hv[:, 0, 0:4], 0.0)
                nc.vector.memset(STA[g][:, 0:4], 0.0)      # s2 = 0
                nc.vector.memset(CS[g][:, 0:1], 0.0)       # guard column

            # ---------------- phase 2: the scan ----------------
            zv = [Z[g][:].rearrange("p (a u) -> p a u", u=UBLK) for g in range(GRP)]
            for t in range(SEQ):
                # emit in ABAB (group-interleaved) order per stage so every
                # dependency sits at instruction distance >= 2
                for g in range(GRP):  # M6
                    hblk = hE[g][:, t * 6:(t + 1) * 6].unsqueeze(1)
                    nc.vector._custom_dve(
                        mulscan,
                        out=CS[g][:, 1:97],
                        in0=hblk.broadcast_to((128, 16, 6)),
                        in1=wxsb[:, 0:96])
                for g in range(GRP):  # DW: th = wrap(ends - starts) -> th slots
                    ends = CS[g][:, 1:97].rearrange("p (m k) -> p m k", k=6)[:, :, 5]
                    starts = CS[g][:, 0:96].rearrange("p (m k) -> p m k", k=6)[:, :, 0]
                    nc.vector._custom_dve(
                        diffwrap,
                        out=zv[g][:, :, 28:32],
                        in0=ends, in1=starts,
                        s0=0.0, s1=PI, imm2=2 * PI)
                for g in range(GRP):  # SIN: z_w at slots 0,6,12,18
                    nc.vector._custom_dve(
                        oddp7,
                        out=zv[g][:, :, 0:24].rearrange("p a (w r) -> p a w r", r=6)[:, :, :, 0],
                        in0=zv[g][:, :, 28:32],
                        in1=cpsb[:, 0:1],
                        s0=sc7[3], s1=sc7[2], imm2=sc7[1])
                for g in range(GRP):  # P1: [q1,m12,b23] = z[0,1,2]*z[1,2,3]
                    nc.vector.tensor_tensor(
                        out=zv[g][:, :, 24:27],
                        in0=zv[g][:, :, 0:18].rearrange("p a (w r) -> p a w r", r=6)[:, :, :, 0],
                        in1=zv[g][:, :, 6:24].rearrange("p a (w r) -> p a w r", r=6)[:, :, :, 0],
                        op=OP.mult)
                for g in range(GRP):  # P2: [q2,q0,q3] = [z2,z3,q1]*[q1,m12,b23]
                    nc.vector.tensor_tensor(
                        out=zv[g][:, :, 21:24],
                        in0=zv[g][:, :, 12:30].rearrange("p a (w r) -> p a w r", r=6)[:, :, :, 0],
                        in1=zv[g][:, :, 24:27],
                        op=OP.mult)
                for g in range(GRP):  # TS: u_a = 1 + tanh(q/2), a in {f,i,o}
                    # in0 = contiguous [q2,q0,q3,q1] window; PERM order
                    nc.vector._custom_dve(
                        tanh5p,
                        out=STB[g][:, 0:12].rearrange("p (a w) -> p a w", w=4),
                        in0=zv[g][:, 0:3, 21:25],
                        s0=tgh[2], s1=tgh[1], imm2=tgh[0])
                for g in range(GRP):  # TU: t_u = tanh(q), a = u
                    nc.vector._custom_dve(
                        tanh5,
                        out=STA[g][:, 4:8],
                        in0=zv[g][:, 3, 21:25],
                        s0=tg[2], s1=tg[1], imm2=tg[0])
                for g in range(GRP):  # AB: [A,B] = [u_f,u_i] * [s2,tu]
                    nc.vector.tensor_tensor(
                        out=STC[g][:].rearrange("p (j w) -> p j w", w=4),
                        in0=STB[g][:, 0:8].rearrange("p (j w) -> p j w", w=4),
                        in1=STA[g][:].rearrange("p (j w) -> p j w", w=4),
                        op=OP.mult)
                for g in range(GRP):  # S2: s2' = 0.5*A + B
                    nc.vector.scalar_tensor_tensor(
                        out=STA[g][:, 0:4], in0=STC[g][:, 0:4], scalar=0.5,
                        in1=STC[g][:, 4:8], op0=OP.mult, op1=OP.add)
                for g in range(GRP):  # HF: h' = u_o * tanh5(s2'/2) -> hE[t+1]
                    nc.vector._custom_dve(
                        hfuse,
                        out=hE[g][:, (t + 1) * 6:(t + 1) * 6 + 4],
                        in0=STA[g][:, 0:4],
                        in1=STB[g][:, 8:12],
                        s0=tch[2], s1=tch[1], imm2=tch[0])

            # ---------------- phase 3: y = h @ (W_out/2) + b_out ----------
            yt = big.tile([128, GRP * SEQ * 4], F32, tag="YT")
            for g in range(GRP):
                hsv = (hE[g][:, 6:].rearrange("p (t k) -> p t k", k=6)[:, :, 0:4])
                wo = (cpsb[:, 4:8].unsqueeze(1).broadcast_to((128, SEQ, 4)))
                nc.vector.tensor_tensor(
                    out=yt[:].rearrange("p (g t w) -> p g t w", g=GRP, t=SEQ)[:, g],
                    in0=hsv, in1=wo, op=OP.mult)
            yy = big.tile([128, GRP * SEQ], F32, tag="Y")
            nc.vector.tensor_reduce(
                out=yy[:], in_=yt[:].rearrange("p (m w) -> p m w", w=4),
                axis=mybir.AxisListType.X, op=OP.add)
            nc.vector.tensor_scalar(out=yy[:], in0=yy[:],
                                    scalar1=cpsb[:, 8:9], scalar2=None,
                                    op0=OP.add)
            nc.sync.dma_start(
                y_d.rearrange("(g p) t -> p g t", p=128),
                yy[:].rearrange("p (g t) -> p g t", g=GRP),
            )
            if debug:
                for g in range(GRP):
                    nc.sync.dma_start(
                        dh_d[:].rearrange("p (g n) -> p g n", g=GRP)[:, g],
                        hE[g][:])

    nc.compile()
    return nc


def _pack_consts(W_f, b_f, W_i, b_i, W_u, b_u, W_o, b_o,
                 rx_f, rx_i, rx_u, rx_o, W_out, b_out):
    """wx[128,96] (m=(a,w), k6=[0.5*Wh permuted, Wx, beta']) and cp[128,16].

    hE h-slots hold h in PERM wire order, so whx's Wh columns are permuted
    to match: whx[m, j] = 0.5*W[a][w_m, 1 + PERM[j]]."""
    Ws = [W_f, W_i, W_o, W_u]          # gate order f,i,o,u
    bs = [b_f, b_i, b_o, b_u]
    rxs = [rx_f, rx_i, rx_o, rx_u]
    whx = np.zeros((4, 4, 6), np.float32)
    for a in range(4):
        Wa = np.asarray(Ws[a], np.float32)
        whx[a, :, 0:4] = 0.5 * Wa[:, 1:5][:, PERM]
        whx[a, :, 4] = Wa[:, 0]
        beta = (np.asarray(bs[a], np.float32)
                + np.asarray(rxs[a], np.float32) + np.float32(np.pi / 2))
        whx[a, :, 5] = (beta + np.pi) % (2 * np.pi) - np.pi
    wx = np.tile(whx.reshape(1, 96), (128, 1)).astype(np.float32)

    cprow = np.zeros(16, np.float32)
    cprow[0] = float(_SIN7[0])                        # sin c3 latch const
    cprow[4:8] = 0.5 * np.asarray(W_out, np.float32)[0][PERM]
    cprow[8] = float(np.asarray(b_out, np.float32)[0])
    cp = np.tile(cprow[None], (128, 1)).astype(np.float32)

    # range check: theta must stay within [-3pi, 3pi] for the single wrap
    whabs = np.abs(whx[:, :, 0:4]).sum(axis=2) * 2.0
    bound = (np.abs(whx[:, :, 5]) + np.abs(whx[:, :, 4]) + whabs).max()
    assert bound < 3 * PI - 0.2, f"theta range {bound} too large for single wrap"
    return wx, cp


def kernel(**inputs):
    from concourse.bass_utils import run_bass_kernel_spmd

    x = np.ascontiguousarray(np.asarray(inputs["x"], np.float32)).reshape(BATCH, SEQ, 4)
    wx, cp = _pack_consts(**{k: v for k, v in inputs.items() if k != "x"})

    if "nc" not in _CACHE:
        _CACHE["nc"] = _build_program()
    nc = _CACHE["nc"]

    in_maps = []
    for cid in range(N_CORES):
        xs = np.ascontiguousarray(
            x[cid * BPC:(cid + 1) * BPC].reshape(BPC, SEQ * 4))
        in_maps.append({"xs": xs, "wx": wx, "cp": cp})

    res = run_bass_kernel_spmd(nc, in_maps, core_ids=list(range(N_CORES)))
    ys = [res.results[cid]["y"] for cid in range(N_CORES)]  # each (BPC, SEQ)
    full = np.concatenate(ys, axis=0)                       # (BATCH, SEQ)
    return np.ascontiguousarray(full.T)[:, :, None].astype(np.float32)


# revision 11
# speedup vs baseline: 1.0259x; 1.0065x over previous
"""Trainium2 Bass kernel for the quantum ConvLSTM reference.

Math reduction (validated to ~7e-3 rel vs the jax reference in numpy sim):
  * quantum_conv(patch) == T16[b] from thresholding the 4 pixels at 127;
    evaluated on-chip as a multilinear polynomial in the 4 bits.
  * qlayer(x, p) == [z1*z2*z3, z0*z1, z0*z1*z2, z0*z1*z2*z3] with
    z_w = cos(x_w + p_w); the LSTM scan becomes a small classical
    recurrence: theta = Wh@h + Wx*conv + beta; z = sin(theta + pi/2);
    gate products; sigmoid via (1+tanh(q/2))/2; c/h updates with the
    2c/2h scaling folded into constants.

Per-step pipeline (10 DVE ops per batch group, 2 groups interleaved
ABAB so every dependency sits at instruction distance >= 2 and the
vector engine's FIFO streams without semaphore stalls):
  M6   one multiply-cumsum of [h0..h3, conv_t, 1] x whx6 (96 els)
       -> full gate preactivation sums via guard-column differences
  DW   custom diff+wrap: wrap(CS[6m+6]-CS[6m]) into [-pi, pi]
  SIN  deg-7 odd sin (8 ALU stages, 4th coef via C3->Src1 latch)
  P1   strided multiply -> [q1, m12, b23] per gate unit
  P2   strided multiply -> [q2, q0, q3] per gate unit
  TS   1 + tanh(q/2) for sigmoid gates (deg-5, 0.5 folded into coefs)
  TU   tanh(q) for the u gate (deg-5)
  AB   paired multiply [u_f*s2, u_i*t_u]
  S2   s2' = 0.5*A + B (stock STT)
  HF   h' = u_o * tanh5(s2'/2), written into the next hE state block

Sharding: pure data parallel over batch (2048 -> 8 cores x 256 rows).
Each core: 128 partitions x 2 column-groups; seq scan of 512 steps fully
unrolled on-chip.
"""

import sys

import numpy as np

sys.path.insert(0, "/opt/trn_rl_repo")

N_CORES = 8
BATCH = 2048
SEQ = 512
BPC = BATCH // N_CORES          # 256 batch rows per core
GRP = BPC // 128                # 2 column groups of 128 partitions
PI = float(np.pi)

# Z-tile slot layout per gate unit (stride UBLK per unit a):
#   z0@0 z1@6 z2@12 z3@18   (sin output, stride 6)
#   q2@21 q0@22 q3@23       (P2 output, stride 1)
#   q1@24 m12@25 b23@26     (P1 output, stride 1)
#   th@28..31               (DW output)
# The tanh stage reads the contiguous window [21..25) = [q2,q0,q3,q1],
# i.e. wire order PERM = [2,0,3,1]. All per-wire state (u, tu, s2, A, B,
# h-slots in hE) is stored in this permuted order; whx's Wh columns and
# W_out are permuted host-side to match, so every AP is a plain
# positive-stride window.
UBLK = 32
PERM = [2, 0, 3, 1]

# STATE-tile slot layout per group:
#   s2@0..3  tu@4..7  u_f@8..11 u_i@12..15 u_o@16..19  A@20..23 B@24..27
SBLK = 32

# ---------------------------------------------------------------------------
# Host-side constants: T16 lookup table + multilinear coefficients.
# ---------------------------------------------------------------------------
_RY_ANGLES = np.random.RandomState(0).uniform(0.0, 2.0 * np.pi, size=(2, 4)).astype(np.float32)


def _build_t16() -> np.ndarray:
    s = np.zeros((16, 2, 2, 2, 2), np.complex64)
    for b in range(16):
        bits = [(b >> 3) & 1, (b >> 2) & 1, (b >> 1) & 1, b & 1]
        s[(b, *bits)] = 1.0

    def ry(state, th, w):
        a0 = np.take(state, 0, axis=1 + w)
        a1 = np.take(state, 1, axis=1 + w)
        c = np.complex64(np.cos(np.float32(th) / 2))
        sn = np.complex64(np.sin(np.float32(th) / 2))
        return np.stack([c * a0 - sn * a1, sn * a0 + c * a1], axis=1 + w)

    def cnot(state, ctl, tgt):
        s0 = np.take(state, 0, axis=1 + ctl)
        s1 = np.take(state, 1, axis=1 + ctl)
        t_ax = 1 + tgt if tgt < ctl else tgt
        s1 = np.flip(s1, axis=t_ax)
        return np.stack([s0, s1], axis=1 + ctl)

    for layer in range(2):
        for w in range(4):
            s = ry(s, _RY_ANGLES[layer, w], w)
        for w in range(3):
            s = cnot(s, w, w + 1)
    probs = np.abs(s) ** 2
    cols = []
    for w in range(4):
        other = tuple(a for a in range(1, 5) if a != 1 + w)
        cols.append(probs.sum(axis=other)[:, 1])
    return np.stack(cols, axis=1).mean(axis=1)  # (16,)


def _multilinear_coeffs(t16: np.ndarray) -> np.ndarray:
    """C[4][4] with T16[b] = sum_jk C[j,k]*u_j*v_k, u=[1,b0,b1,b0b1], v=[1,b2,b3,b2b3]."""
    m = np.zeros((16, 16))
    for b in range(16):
        b0, b1, b2, b3 = (b >> 3) & 1, (b >> 2) & 1, (b >> 1) & 1, b & 1
        u = [1, b0, b1, b0 * b1]
        v = [1, b2, b3, b2 * b3]
        for j in range(4):
            for k in range(4):
                m[b, j * 4 + k] = u[j] * v[k]
    return np.linalg.solve(m, t16.astype(np.float64)).reshape(4, 4)


_T16 = _build_t16()
_CML = _multilinear_coeffs(_T16)


def _fit_odd(f, hi, ncoef):
    """Near-minimax odd fit f(x) ~ x*p(x^2) on [-hi, hi]; p coeffs ascending."""
    n = 4000
    k = np.arange(n)
    x = np.cos(np.pi * (k + 0.5) / n) * hi
    y = x * x
    a = np.stack([x * y ** j for j in range(ncoef)], axis=1)
    tgt = f(x)
    c, *_ = np.linalg.lstsq(a, tgt, rcond=None)
    for _ in range(60):
        r = a @ c - tgt
        w = (np.abs(r) + 1e-12) ** 0.5
        c, *_ = np.linalg.lstsq(a * w[:, None], tgt * w, rcond=None)
    return c


_SIN7 = _fit_odd(np.sin, np.pi, 4)       # deg-7 odd sin on [-pi, pi], ~2.6e-4
# Pre-scale x by 1/lambda so the leading coefficient becomes exactly 1.0 and
# the sin op needs only 3 constants (no C3 latch): sin(x) ~ x'*(c0'+c1'y'+c2'y'^2+y'^3)
_SLAM = -np.abs(1.0 / _SIN7[3]) ** (1 / 7) if _SIN7[3] < 0 else (1.0 / _SIN7[3]) ** (1 / 7)
_SIN7S = [float(_SIN7[k] * _SLAM ** (2 * k + 1)) for k in range(4)]
_TG5 = _fit_odd(np.tanh, 1.0, 3)         # deg-5 tanh on [-1, 1], ~4e-4
_TC5 = _fit_odd(np.tanh, 2.0, 3)         # deg-5 tanh on [-2, 2] for tanh(c)
# tanh(q/2) coefs: c_k' = c_k / 2^(2k+1)
_TG5H = _TG5 * np.array([1 / 2 ** (2 * k + 1) for k in range(3)])
# tanh(s2/2) coefs for HFUSE (s2 = 2c)
_TC5H = _TC5 * np.array([1 / 2 ** (2 * k + 1) for k in range(3)])

_CACHE = {}


def _register_custom_ops():
    """Register fused DVE ops (idempotent). Shas are pinned by bootstrap:
    compile once with an empty pin, parse the actual sha from the error."""
    import re
    import concourse.dve_ops as dve_ops_mod
    from concourse.dve_ops import OPS, DveOp
    from concourse.dve_spec import (
        Spec, Src0, Src1, C0, C1, C2, Zero, One, Latch, scan, AluOp, sq,
    )

    have = {o.name for o in OPS}

    def make(name, spec):
        if name in have:
            return next(o for o in OPS if o.name == name)
        probe = DveOp(name, spec, subdim=False, uops_sha={})
        OPS.append(probe)
        dve_ops_mod._SUB_OPCODE_FOR_NAME[name] = (
            dve_ops_mod._CUSTOM_DVE_ROW_BASE + len(OPS) - 1)
        shas = {}
        for ver in ("v3", "v4"):
            try:
                probe.compile(ver)
            except ValueError as e:
                mm = re.search(r"(\b[0-9a-f]{16})\b", str(e))
                shas[ver] = mm.group(1)
        OPS.remove(probe)
        op = DveOp(name, spec, subdim=False, uops_sha=shas)
        OPS.append(op)
        return op

    # out = cumsum(in0 * in1) along the free stream (same as baseline)
    def _ref_mulscan(in0, in1, c0, c1, c2):
        p = (in0.astype(np.float32) * in1).reshape(in0.shape[0], -1)
        return np.cumsum(p, axis=1, dtype=np.float32).reshape(in0.shape)

    mulscan = make("MULSCAN_ANT", Spec(
        body=scan(AluOp.ADD, Src0 * Src1),
        reference=_ref_mulscan))

    # out = c0 * wrap(in0 - in1) into [-c1, c1] by one period c2
    d = Src0 - Src1
    dws = (d + C2 * ((d < (Zero - C1)) - (C1 < d))) * C0
    def _ref_dws(in0, in1, c0, c1, c2):
        dd = (in0.astype(np.float32) - in1).astype(np.float32)
        return (c0 * (dd + c2 * ((dd < -c1).astype(np.float32)
                                 - (c1 < dd).astype(np.float32)))).astype(np.float32)
    diffwrap = make("DIFFWRAPS_ANT", Spec(body=dws, reference=_ref_dws))

    # out = in0 - in1 + c0  (phase-3 segment-sum extraction + bias)
    da = Src0 - Src1 + C0
    def _ref_da(in0, in1, c0, c1, c2):
        return (in0.astype(np.float32) - in1 + c0).astype(np.float32)
    diffadd = make("DIFFADD_ANT", Spec(body=da, reference=_ref_da))

    # out = x*(((y + c2)*y + c1)*y + c0), y = x^2 (deg-7 odd, monic leading)
    y7 = sq(Src0)
    p7 = (((y7 + C2) * y7 + C1) * y7 + C0) * Src0
    def _ref_p7(in0, in1, c0, c1, c2):
        yy = in0.astype(np.float32) ** 2
        return ((((yy + c2) * yy + c1) * yy + c0) * in0).astype(np.float32)
    oddp7 = make("ODDP7N_ANT", Spec(body=p7, reference=_ref_p7))

    # out = ((c0*y + c1)*y + c2)*x + 1, y = x^2  (deg-5 odd + 1)
    y5 = sq(Src0)
    t5p = ((C0 * y5 + C1) * y5 + C2) * Src0 + One
    def _ref_t5p(in0, in1, c0, c1, c2):
        yy = in0.astype(np.float32) ** 2
        return (((c0 * yy + c1) * yy + c2) * in0 + 1.0).astype(np.float32)
    tanh5p = make("TANH5P_ANT", Spec(body=t5p, reference=_ref_t5p))

    # out = ((c0*y + c1)*y + c2)*x, y = x^2  (deg-5 odd)
    t5 = ((C0 * y5 + C1) * y5 + C2) * Src0
    def _ref_t5(in0, in1, c0, c1, c2):
        yy = in0.astype(np.float32) ** 2
        return (((c0 * yy + c1) * yy + c2) * in0).astype(np.float32)
    tanh5 = make("TANH5_ANT", Spec(body=t5, reference=_ref_t5))

    # out = in1 * ((c0*y + c1)*y + c2)*in0, y = in0^2  (h update)
    hf = Src1 * (((C0 * y5 + C1) * y5 + C2) * Src0)
    def _ref_hf(in0, in1, c0, c1, c2):
        yy = in0.astype(np.float32) ** 2
        return (in1 * (((c0 * yy + c1) * yy + c2) * in0)).astype(np.float32)
    hfuse = make("HFUSE_ANT", Spec(body=hf, reference=_ref_hf))

    return mulscan, diffwrap, diffadd, oddp7, tanh5p, tanh5, hfuse


def _build_program(debug=False):
    """Build + compile the (weights-independent) single-core SPMD Bass program."""
    import concourse.bass as bass
    import concourse.mybir as mybir
    import concourse.tile as tile
    from concourse import bacc

    F32 = mybir.dt.float32
    OP = mybir.AluOpType

    mulscan, diffwrap, diffadd, oddp7, tanh5p, tanh5, hfuse = _register_custom_ops()

    nc = bacc.Bacc(None, target_bir_lowering=False)

    x_d = nc.dram_tensor("xs", [BPC, SEQ * 4], F32, kind="ExternalInput")
    wx_d = nc.dram_tensor("wx", [128, 96], F32, kind="ExternalInput")
    cp_d = nc.dram_tensor("cp", [128, 16], F32, kind="ExternalInput")
    y_d = nc.dram_tensor("y", [BPC, SEQ], F32, kind="ExternalOutput")
    if debug:
        dh_d = nc.dram_tensor("dbg_h", [128, GRP * (SEQ + 1) * 6], F32, kind="ExternalOutput")

    sc7 = [float(v) for v in _SIN7]
    tgh = [float(v) for v in _TG5H]
    tg = [float(v) for v in _TG5]
    tch = [float(v) for v in _TC5H]

    with tile.TileContext(nc) as tc:
        with (
            tc.tile_pool(name="big", bufs=1) as big,
            tc.tile_pool(name="ph1", bufs=1) as ph1,
        ):
            # ---------------- load ----------------
            xsb = big.tile([128, GRP * SEQ * 4], F32, tag="X")       # (g, t, k)
            nc.sync.dma_start(
                xsb[:].rearrange("p (g n) -> p g n", g=GRP),
                x_d.rearrange("(g p) n -> p g n", p=128),
            )
            wxsb = big.tile([128, 96], F32, tag="WX")                # (a, w, k6)
            nc.sync.dma_start(wxsb[:], wx_d[:])
            cpsb = big.tile([128, 16], F32, tag="CP")
            nc.sync.dma_start(cpsb[:], cp_d[:])

            # ---------------- phase 1: bits -> conv ----------------
            bsb = big.tile([128, GRP * SEQ * 4], F32, tag="B")
            nc.vector.tensor_scalar(out=bsb[:], in0=xsb[:], scalar1=127.0,
                                    scalar2=None, op0=OP.is_gt)
            bv = bsb[:].rearrange("p (g t k) -> p g t k", g=GRP, k=4)
            bk = [bv[:, :, :, k] for k in range(4)]                  # each (128, g, t)

            q01 = ph1.tile([128, GRP * SEQ], F32, tag="q01")
            q23 = ph1.tile([128, GRP * SEQ], F32, tag="q23")
            gt = lambda tl: tl[:].rearrange("p (g t) -> p g t", g=GRP)
            nc.vector.tensor_tensor(out=gt(q01), in0=bk[0], in1=bk[1], op=OP.mult)
            nc.vector.tensor_tensor(out=gt(q23), in0=bk[2], in1=bk[3], op=OP.mult)
            rs = []
            for j in range(4):
                r = ph1.tile([128, GRP * SEQ], F32, tag=f"r{j}")
                nc.vector.tensor_scalar(out=gt(r), in0=bk[2],
                                        scalar1=float(_CML[j, 1]),
                                        scalar2=float(_CML[j, 0]),
                                        op0=OP.mult, op1=OP.add)
                nc.vector.affine_then_add(out=gt(r), in0=bk[3], in1=gt(r),
                                          scale=float(_CML[j, 2]), bias=0.0)
                nc.vector.affine_then_add(out=gt(r), in0=gt(q23), in1=gt(r),
                                          scale=float(_CML[j, 3]), bias=0.0)
                rs.append(r)
            m = ph1.tile([128, GRP * SEQ], F32, tag="m")
            vcv = big.tile([128, GRP * SEQ], F32, tag="V")           # conv (g, t)
            nc.vector.tensor_tensor(out=gt(m), in0=bk[0], in1=gt(rs[1]), op=OP.mult)
            nc.vector.tensor_tensor(out=gt(vcv), in0=gt(rs[0]), in1=gt(m), op=OP.add)
            nc.vector.tensor_tensor(out=gt(m), in0=bk[1], in1=gt(rs[2]), op=OP.mult)
            nc.vector.tensor_tensor(out=gt(vcv), in0=gt(vcv), in1=gt(m), op=OP.add)
            nc.vector.tensor_tensor(out=gt(m), in0=gt(q01), in1=gt(rs[3]), op=OP.mult)
            nc.vector.tensor_tensor(out=gt(vcv), in0=gt(vcv), in1=gt(m), op=OP.add)

            # ---------------- phase 1b: populate hE state buffers ----------
            # hE_g block t (cols t*6 .. t*6+5) = [h0..h3, conv_t, 1.0]
            hE = [big.tile([128, (SEQ + 1) * 6], F32, tag=f"HE{g}", name=f"HE{g}") for g in range(GRP)]
            CS = [big.tile([128, 97], F32, tag=f"CS{g}", name=f"CS{g}") for g in range(GRP)]
            Z = [big.tile([128, 4 * UBLK], F32, tag=f"Z{g}", name=f"Z{g}") for g in range(GRP)]
            STA = [big.tile([128, 8], F32, tag=f"STA{g}", name=f"STA{g}") for g in range(GRP)]
            STB = [big.tile([128, 12], F32, tag=f"STB{g}", name=f"STB{g}") for g in range(GRP)]
            STC = [big.tile([128, 8], F32, tag=f"STC{g}", name=f"STC{g}") for g in range(GRP)]

            for g in range(GRP):
                hv = hE[g][:].rearrange("p (t k) -> p t k", k=6)
                # conv slots: hE[t*6+4] = vcv[g, t]
                nc.vector.tensor_scalar(
                    out=hv[:, 0:SEQ, 4], in0=vcv[:].rearrange(
                        "p (g t) -> p g t", g=GRP)[:, g, :],
                    scalar1=1.0, scalar2=None, op0=OP.mult)
                nc.vector.memset(hv[:, 0:SEQ, 5], 1.0)
                nc.vector.memset(hv[:, 0, 0:4], 0.0)
                nc.vector.memset(STA[g][:, 0:4], 0.0)      # s2 = 0
                nc.vector.memset(CS[g][:, 0:1], 0.0)       # guard column

            # ---------------- phase 2: the scan ----------------
            zv = [Z[g][:].rearrange("p (a u) -> p a u", u=UBLK) for g in range(GRP)]
            for t in range(SEQ):
                # emit in ABAB (group-interleaved) order per stage so every
                # dependency sits at instruction distance >= 2
                for g in range(GRP):  # M6
                    hblk = hE[g][:, t * 6:(t + 1) * 6].unsqueeze(1)
                    nc.vector._custom_dve(
                        mulscan,
                        out=CS[g][:, 1:97],
                        in0=hblk.broadcast_to((128, 16, 6)),
                        in1=wxsb[:, 0:96])
                for g in range(GRP):  # DW: th = wrap(ends - starts) -> th slots
                    ends = CS[g][:, 1:97].rearrange("p (m k) -> p m k", k=6)[:, :, 5]
                    starts = CS[g][:, 0:96].rearrange("p (m k) -> p m k", k=6)[:, :, 0]
                    nc.vector._custom_dve(
                        diffwrap,
                        out=zv[g][:, :, 28:32],
                        in0=ends, in1=starts,
                        s0=float(1.0 / _SLAM), s1=PI, imm2=2 * PI)
                for g in range(GRP):  # SIN: z_w at slots 0,6,12,18
                    nc.vector._custom_dve(
                        oddp7,
                        out=zv[g][:, :, 0:24].rearrange("p a (w r) -> p a w r", r=6)[:, :, :, 0],
                        in0=zv[g][:, :, 28:32],
                        s0=_SIN7S[0], s1=_SIN7S[1], imm2=_SIN7S[2])
                for g in range(GRP):  # P1: [q1,m12,b23] = z[0,1,2]*z[1,2,3]
                    nc.vector.tensor_tensor(
                        out=zv[g][:, :, 24:27],
                        in0=zv[g][:, :, 0:18].rearrange("p a (w r) -> p a w r", r=6)[:, :, :, 0],
                        in1=zv[g][:, :, 6:24].rearrange("p a (w r) -> p a w r", r=6)[:, :, :, 0],
                        op=OP.mult)
                for g in range(GRP):  # P2: [q2,q0,q3] = [z2,z3,q1]*[q1,m12,b23]
                    nc.vector.tensor_tensor(
                        out=zv[g][:, :, 21:24],
                        in0=zv[g][:, :, 12:30].rearrange("p a (w r) -> p a w r", r=6)[:, :, :, 0],
                        in1=zv[g][:, :, 24:27],
                        op=OP.mult)
                for g in range(GRP):  # TS: u_a = 1 + tanh(q/2), a in {f,i,o}
                    # in0 = contiguous [q2,q0,q3,q1] window; PERM order
                    nc.vector._custom_dve(
                        tanh5p,
                        out=STB[g][:, 0:12].rearrange("p (a w) -> p a w", w=4),
                        in0=zv[g][:, 0:3, 21:25],
                        s0=tgh[2], s1=tgh[1], imm2=tgh[0])
                for g in range(GRP):  # TU: t_u = tanh(q), a = u
                    nc.vector._custom_dve(
                        tanh5,
                        out=STA[g][:, 4:8],
                        in0=zv[g][:, 3, 21:25],
                        s0=tg[2], s1=tg[1], imm2=tg[0])
                for g in range(GRP):  # AB: [A,B] = [u_f,u_i] * [s2,tu]
                    nc.vector.tensor_tensor(
                        out=STC[g][:].rearrange("p (j w) -> p j w", w=4),
                        in0=STB[g][:, 0:8].rearrange("p (j w) -> p j w", w=4),
                        in1=STA[g][:].rearrange("p (j w) -> p j w", w=4),
                        op=OP.mult)
                for g in range(GRP):  # S2: s2' = 0.5*A + B
                    nc.vector.scalar_tensor_tensor(
                        out=STA[g][:, 0:4], in0=STC[g][:, 0:4], scalar=0.5,
                        in1=STC[g][:, 4:8], op0=OP.mult, op1=OP.add)
                for g in range(GRP):  # HF: h' = u_o * tanh5(s2'/2) -> hE[t+1]
                    nc.vector._custom_dve(
                        hfuse,
                        out=hE[g][:, (t + 1) * 6:(t + 1) * 6 + 4],
                        in0=STA[g][:, 0:4],
                        in1=STB[g][:, 8:12],
                        s0=tch[2], s1=tch[1], imm2=tch[0])

            # ---------------- phase 3: y = h @ (W_out/2) + b_out ----------
            # cumsum of h*wo along (t, w), then segment sums via guard diffs
            YC = [big.tile([128, SEQ * 4 + 1], F32, tag=f"YC{g}", name=f"YC{g}")
                  for g in range(GRP)]
            yy = big.tile([128, GRP * SEQ], F32, tag="Y")
            for g in range(GRP):
                nc.vector.memset(YC[g][:, 0:1], 0.0)
            for g in range(GRP):
                hsv = (hE[g][:, 6:].rearrange("p (t k) -> p t k", k=6)[:, :, 0:4])
                wo = (cpsb[:, 4:8].unsqueeze(1).broadcast_to((128, SEQ, 4)))
                nc.vector._custom_dve(
                    mulscan,
                    out=YC[g][:, 1:SEQ * 4 + 1],
                    in0=hsv, in1=wo)
            for g in range(GRP):
                yends = YC[g][:, 1:SEQ * 4 + 1].rearrange(
                    "p (m k) -> p m k", k=4)[:, :, 3]
                ystarts = YC[g][:, 0:SEQ * 4].rearrange(
                    "p (m k) -> p m k", k=4)[:, :, 0]
                nc.vector._custom_dve(
                    diffadd,
                    out=yy[:].rearrange("p (g t) -> p g t", g=GRP)[:, g],
                    in0=yends, in1=ystarts,
                    s0=cpsb[:, 8:9])
            nc.sync.dma_start(
                y_d.rearrange("(g p) t -> p g t", p=128),
                yy[:].rearrange("p (g t) -> p g t", g=GRP),
            )
            if debug:
                for g in range(GRP):
                    nc.sync.dma_start(
                        dh_d[:].rearrange("p (g n) -> p g n", g=GRP)[:, g],
                        hE[g][:])

    nc.compile()
    return nc


def _pack_consts(W_f, b_f, W_i, b_i, W_u, b_u, W_o, b_o,
                 rx_f, rx_i, rx_u, rx_o, W_out, b_out):
    """wx[128,96] (m=(a,w), k6=[0.5*Wh permuted, Wx, beta']) and cp[128,16].

    hE h-slots hold h in PERM wire order, so whx's Wh columns are permuted
    to match: whx[m, j] = 0.5*W[a][w_m, 1 + PERM[j]]."""
    Ws = [W_f, W_i, W_o, W_u]          # gate order f,i,o,u
    bs = [b_f, b_i, b_o, b_u]
    rxs = [rx_f, rx_i, rx_o, rx_u]
    whx = np.zeros((4, 4, 6), np.float32)
    for a in range(4):
        Wa = np.asarray(Ws[a], np.float32)
        whx[a, :, 0:4] = 0.5 * Wa[:, 1:5][:, PERM]
        whx[a, :, 4] = Wa[:, 0]
        beta = (np.asarray(bs[a], np.float32)
                + np.asarray(rxs[a], np.float32) + np.float32(np.pi / 2))
        whx[a, :, 5] = (beta + np.pi) % (2 * np.pi) - np.pi
    wx = np.tile(whx.reshape(1, 96), (128, 1)).astype(np.float32)

    cprow = np.zeros(16, np.float32)
    cprow[0] = float(_SIN7[0])                        # sin c3 latch const
    cprow[4:8] = 0.5 * np.asarray(W_out, np.float32)[0][PERM]
    cprow[8] = float(np.asarray(b_out, np.float32)[0])
    cp = np.tile(cprow[None], (128, 1)).astype(np.float32)

    # range check: theta must stay within [-3pi, 3pi] for the single wrap
    whabs = np.abs(whx[:, :, 0:4]).sum(axis=2) * 2.0
    bound = (np.abs(whx[:, :, 5]) + np.abs(whx[:, :, 4]) + whabs).max()
    assert bound < 3 * PI - 0.2, f"theta range {bound} too large for single wrap"
    return wx, cp


def kernel(**inputs):
    from concourse.bass_utils import run_bass_kernel_spmd

    x = np.ascontiguousarray(np.asarray(inputs["x"], np.float32)).reshape(BATCH, SEQ, 4)
    wx, cp = _pack_consts(**{k: v for k, v in inputs.items() if k != "x"})

    if "nc" not in _CACHE:
        _CACHE["nc"] = _build_program()
    nc = _CACHE["nc"]

    in_maps = []
    for cid in range(N_CORES):
        xs = np.ascontiguousarray(
            x[cid * BPC:(cid + 1) * BPC].reshape(BPC, SEQ * 4))
        in_maps.append({"xs": xs, "wx": wx, "cp": cp})

    res = run_bass_kernel_spmd(nc, in_maps, core_ids=list(range(N_CORES)))
    ys = [res.results[cid]["y"] for cid in range(N_CORES)]  # each (BPC, SEQ)
    full = np.concatenate(ys, axis=0)                       # (BATCH, SEQ)
    return np.ascontiguousarray(full.T)[:, :, None].astype(np.float32)


# revision 12
# speedup vs baseline: 1.0275x; 1.0016x over previous
"""Trainium2 Bass kernel for the quantum ConvLSTM reference.

Math reduction (validated to ~7e-3 rel vs the jax reference in numpy sim):
  * quantum_conv(patch) == T16[b] from thresholding the 4 pixels at 127;
    evaluated on-chip as a multilinear polynomial in the 4 bits.
  * qlayer(x, p) == [z1*z2*z3, z0*z1, z0*z1*z2, z0*z1*z2*z3] with
    z_w = cos(x_w + p_w); the LSTM scan becomes a small classical
    recurrence: theta = Wh@h + Wx*conv + beta; z = sin(theta + pi/2);
    gate products; sigmoid via (1+tanh(q/2))/2; c/h updates with the
    2c/2h scaling folded into constants.

Per-step pipeline (10 DVE ops per batch group, 2 groups interleaved
ABAB so every dependency sits at instruction distance >= 2 and the
vector engine's FIFO streams without semaphore stalls):
  M6   one multiply-cumsum of [h0..h3, conv_t, 1] x whx6 (96 els)
       -> full gate preactivation sums via guard-column differences
  DW   custom diff+wrap: wrap(CS[6m+6]-CS[6m]) into [-pi, pi]
  SIN  deg-7 odd sin (8 ALU stages, 4th coef via C3->Src1 latch)
  P1   strided multiply -> [q1, m12, b23] per gate unit
  P2   strided multiply -> [q2, q0, q3] per gate unit
  TS   1 + tanh(q/2) for sigmoid gates (deg-5, 0.5 folded into coefs)
  TU   tanh(q) for the u gate (deg-5)
  AB   paired multiply [u_f*s2, u_i*t_u]
  S2   s2' = 0.5*A + B (stock STT)
  HF   h' = u_o * tanh5(s2'/2), written into the next hE state block

Sharding: pure data parallel over batch (2048 -> 8 cores x 256 rows).
Each core: 128 partitions x 2 column-groups; seq scan of 512 steps fully
unrolled on-chip.
"""

import sys

import numpy as np

sys.path.insert(0, "/opt/trn_rl_repo")

N_CORES = 8
BATCH = 2048
SEQ = 512
BPC = BATCH // N_CORES          # 256 batch rows per core
GRP = BPC // 128                # 2 column groups of 128 partitions
PI = float(np.pi)

# Z-tile slot layout per gate unit (stride UBLK per unit a):
#   z0@0 z1@6 z2@12 z3@18   (sin output, stride 6)
#   q2@21 q0@22 q3@23       (P2 output, stride 1)
#   q1@24 m12@25 b23@26     (P1 output, stride 1)
#   th@28..31               (DW output)
# The tanh stage reads the contiguous window [21..25) = [q2,q0,q3,q1],
# i.e. wire order PERM = [2,0,3,1]. All per-wire state (u, tu, s2, A, B,
# h-slots in hE) is stored in this permuted order; whx's Wh columns and
# W_out are permuted host-side to match, so every AP is a plain
# positive-stride window.
UBLK = 32
PERM = [2, 0, 3, 1]

# STATE-tile slot layout per group:
#   s2@0..3  tu@4..7  u_f@8..11 u_i@12..15 u_o@16..19  A@20..23 B@24..27
SBLK = 32

# ---------------------------------------------------------------------------
# Host-side constants: T16 lookup table + multilinear coefficients.
# ---------------------------------------------------------------------------
_RY_ANGLES = np.random.RandomState(0).uniform(0.0, 2.0 * np.pi, size=(2, 4)).astype(np.float32)


def _build_t16() -> np.ndarray:
    s = np.zeros((16, 2, 2, 2, 2), np.complex64)
    for b in range(16):
        bits = [(b >> 3) & 1, (b >> 2) & 1, (b >> 1) & 1, b & 1]
        s[(b, *bits)] = 1.0

    def ry(state, th, w):
        a0 = np.take(state, 0, axis=1 + w)
        a1 = np.take(state, 1, axis=1 + w)
        c = np.complex64(np.cos(np.float32(th) / 2))
        sn = np.complex64(np.sin(np.float32(th) / 2))
        return np.stack([c * a0 - sn * a1, sn * a0 + c * a1], axis=1 + w)

    def cnot(state, ctl, tgt):
        s0 = np.take(state, 0, axis=1 + ctl)
        s1 = np.take(state, 1, axis=1 + ctl)
        t_ax = 1 + tgt if tgt < ctl else tgt
        s1 = np.flip(s1, axis=t_ax)
        return np.stack([s0, s1], axis=1 + ctl)

    for layer in range(2):
        for w in range(4):
            s = ry(s, _RY_ANGLES[layer, w], w)
        for w in range(3):
            s = cnot(s, w, w + 1)
    probs = np.abs(s) ** 2
    cols = []
    for w in range(4):
        other = tuple(a for a in range(1, 5) if a != 1 + w)
        cols.append(probs.sum(axis=other)[:, 1])
    return np.stack(cols, axis=1).mean(axis=1)  # (16,)


def _multilinear_coeffs(t16: np.ndarray) -> np.ndarray:
    """C[4][4] with T16[b] = sum_jk C[j,k]*u_j*v_k, u=[1,b0,b1,b0b1], v=[1,b2,b3,b2b3]."""
    m = np.zeros((16, 16))
    for b in range(16):
        b0, b1, b2, b3 = (b >> 3) & 1, (b >> 2) & 1, (b >> 1) & 1, b & 1
        u = [1, b0, b1, b0 * b1]
        v = [1, b2, b3, b2 * b3]
        for j in range(4):
            for k in range(4):
                m[b, j * 4 + k] = u[j] * v[k]
    return np.linalg.solve(m, t16.astype(np.float64)).reshape(4, 4)


_T16 = _build_t16()
_CML = _multilinear_coeffs(_T16)


def _fit_odd(f, hi, ncoef):
    """Near-minimax odd fit f(x) ~ x*p(x^2) on [-hi, hi]; p coeffs ascending."""
    n = 4000
    k = np.arange(n)
    x = np.cos(np.pi * (k + 0.5) / n) * hi
    y = x * x
    a = np.stack([x * y ** j for j in range(ncoef)], axis=1)
    tgt = f(x)
    c, *_ = np.linalg.lstsq(a, tgt, rcond=None)
    for _ in range(60):
        r = a @ c - tgt
        w = (np.abs(r) + 1e-12) ** 0.5
        c, *_ = np.linalg.lstsq(a * w[:, None], tgt * w, rcond=None)
    return c


_SIN7 = _fit_odd(np.sin, np.pi, 4)       # deg-7 odd sin on [-pi, pi], ~2.6e-4
# Pre-scale x by 1/lambda so the leading coefficient becomes exactly 1.0 and
# the sin op needs only 3 constants (no C3 latch): sin(x) ~ x'*(c0'+c1'y'+c2'y'^2+y'^3)
_SLAM = -np.abs(1.0 / _SIN7[3]) ** (1 / 7) if _SIN7[3] < 0 else (1.0 / _SIN7[3]) ** (1 / 7)
_SIN7S = [float(_SIN7[k] * _SLAM ** (2 * k + 1)) for k in range(4)]
_TG5 = _fit_odd(np.tanh, 1.0, 3)         # deg-5 tanh on [-1, 1], ~4e-4
_TC5 = _fit_odd(np.tanh, 2.0, 3)         # deg-5 tanh on [-2, 2] for tanh(c)
# tanh(q/2) coefs: c_k' = c_k / 2^(2k+1)
_TG5H = _TG5 * np.array([1 / 2 ** (2 * k + 1) for k in range(3)])
# tanh(s2/2) coefs for HFUSE (s2 = 2c)
_TC5H = _TC5 * np.array([1 / 2 ** (2 * k + 1) for k in range(3)])

_CACHE = {}


def _register_custom_ops():
    """Register fused DVE ops (idempotent). Shas are pinned by bootstrap:
    compile once with an empty pin, parse the actual sha from the error."""
    import re
    import concourse.dve_ops as dve_ops_mod
    from concourse.dve_ops import OPS, DveOp
    from concourse.dve_spec import (
        Spec, Src0, Src1, C0, C1, C2, Zero, One, Latch, scan, AluOp, sq,
    )

    have = {o.name for o in OPS}

    def make(name, spec):
        if name in have:
            return next(o for o in OPS if o.name == name)
        probe = DveOp(name, spec, subdim=False, uops_sha={})
        OPS.append(probe)
        dve_ops_mod._SUB_OPCODE_FOR_NAME[name] = (
            dve_ops_mod._CUSTOM_DVE_ROW_BASE + len(OPS) - 1)
        shas = {}
        for ver in ("v3", "v4"):
            try:
                probe.compile(ver)
            except ValueError as e:
                mm = re.search(r"(\b[0-9a-f]{16})\b", str(e))
                shas[ver] = mm.group(1)
        OPS.remove(probe)
        op = DveOp(name, spec, subdim=False, uops_sha=shas)
        OPS.append(op)
        return op

    # out = cumsum(in0 * in1) along the free stream (same as baseline)
    def _ref_mulscan(in0, in1, c0, c1, c2):
        p = (in0.astype(np.float32) * in1).reshape(in0.shape[0], -1)
        return np.cumsum(p, axis=1, dtype=np.float32).reshape(in0.shape)

    mulscan = make("MULSCAN_ANT", Spec(
        body=scan(AluOp.ADD, Src0 * Src1),
        reference=_ref_mulscan))

    # out = c0 * wrap(in0 - in1) into [-c1, c1] by one period c2
    d = Src0 - Src1
    dws = (d + C2 * ((d < (Zero - C1)) - (C1 < d))) * C0
    def _ref_dws(in0, in1, c0, c1, c2):
        dd = (in0.astype(np.float32) - in1).astype(np.float32)
        return (c0 * (dd + c2 * ((dd < -c1).astype(np.float32)
                                 - (c1 < dd).astype(np.float32)))).astype(np.float32)
    diffwrap = make("DIFFWRAPS_ANT", Spec(body=dws, reference=_ref_dws))

    # out = in0 - in1 + c0  (phase-3 segment-sum extraction + bias)
    da = Src0 - Src1 + C0
    def _ref_da(in0, in1, c0, c1, c2):
        return (in0.astype(np.float32) - in1 + c0).astype(np.float32)
    diffadd = make("DIFFADD_ANT", Spec(body=da, reference=_ref_da))

    # out = c0 + c1*in0 + c2*in1  (phase-1 multilinear partial)
    rs2b = C0 + C1 * Src0 + C2 * Src1
    def _ref_rs2(in0, in1, c0, c1, c2):
        return (c0 + c1 * in0.astype(np.float32) + c2 * in1).astype(np.float32)
    rs2 = make("RS2_ANT", Spec(body=rs2b, reference=_ref_rs2))

    # out = x*(((y + c2)*y + c1)*y + c0), y = x^2 (deg-7 odd, monic leading)
    y7 = sq(Src0)
    p7 = (((y7 + C2) * y7 + C1) * y7 + C0) * Src0
    def _ref_p7(in0, in1, c0, c1, c2):
        yy = in0.astype(np.float32) ** 2
        return ((((yy + c2) * yy + c1) * yy + c0) * in0).astype(np.float32)
    oddp7 = make("ODDP7N_ANT", Spec(body=p7, reference=_ref_p7))

    # out = ((c0*y + c1)*y + c2)*x + 1, y = x^2  (deg-5 odd + 1)
    y5 = sq(Src0)
    t5p = ((C0 * y5 + C1) * y5 + C2) * Src0 + One
    def _ref_t5p(in0, in1, c0, c1, c2):
        yy = in0.astype(np.float32) ** 2
        return (((c0 * yy + c1) * yy + c2) * in0 + 1.0).astype(np.float32)
    tanh5p = make("TANH5P_ANT", Spec(body=t5p, reference=_ref_t5p))

    # out = ((c0*y + c1)*y + c2)*x, y = x^2  (deg-5 odd)
    t5 = ((C0 * y5 + C1) * y5 + C2) * Src0
    def _ref_t5(in0, in1, c0, c1, c2):
        yy = in0.astype(np.float32) ** 2
        return (((c0 * yy + c1) * yy + c2) * in0).astype(np.float32)
    tanh5 = make("TANH5_ANT", Spec(body=t5, reference=_ref_t5))

    # out = in1 * ((c0*y + c1)*y + c2)*in0, y = in0^2  (h update)
    hf = Src1 * (((C0 * y5 + C1) * y5 + C2) * Src0)
    def _ref_hf(in0, in1, c0, c1, c2):
        yy = in0.astype(np.float32) ** 2
        return (in1 * (((c0 * yy + c1) * yy + c2) * in0)).astype(np.float32)
    hfuse = make("HFUSE_ANT", Spec(body=hf, reference=_ref_hf))

    return mulscan, diffwrap, diffadd, rs2, oddp7, tanh5p, tanh5, hfuse


def _build_program(debug=False):
    """Build + compile the (weights-independent) single-core SPMD Bass program."""
    import concourse.bass as bass
    import concourse.mybir as mybir
    import concourse.tile as tile
    from concourse import bacc

    F32 = mybir.dt.float32
    OP = mybir.AluOpType

    mulscan, diffwrap, diffadd, rs2, oddp7, tanh5p, tanh5, hfuse = _register_custom_ops()

    nc = bacc.Bacc(None, target_bir_lowering=False)

    x_d = nc.dram_tensor("xs", [BPC, SEQ * 4], F32, kind="ExternalInput")
    wx_d = nc.dram_tensor("wx", [128, 96], F32, kind="ExternalInput")
    cp_d = nc.dram_tensor("cp", [128, 16], F32, kind="ExternalInput")
    y_d = nc.dram_tensor("y", [BPC, SEQ], F32, kind="ExternalOutput")
    if debug:
        dh_d = nc.dram_tensor("dbg_h", [128, GRP * (SEQ + 1) * 6], F32, kind="ExternalOutput")

    sc7 = [float(v) for v in _SIN7]
    tgh = [float(v) for v in _TG5H]
    tg = [float(v) for v in _TG5]
    tch = [float(v) for v in _TC5H]

    with tile.TileContext(nc) as tc:
        with (
            tc.tile_pool(name="big", bufs=1) as big,
            tc.tile_pool(name="ph1", bufs=1) as ph1,
        ):
            # ---------------- load ----------------
            xsb = big.tile([128, GRP * SEQ * 4], F32, tag="X")       # (g, t, k)
            nc.sync.dma_start(
                xsb[:].rearrange("p (g n) -> p g n", g=GRP),
                x_d.rearrange("(g p) n -> p g n", p=128),
            )
            wxsb = big.tile([128, 96], F32, tag="WX")                # (a, w, k6)
            nc.sync.dma_start(wxsb[:], wx_d[:])
            cpsb = big.tile([128, 16], F32, tag="CP")
            nc.sync.dma_start(cpsb[:], cp_d[:])

            # ---------------- phase 1: bits -> conv ----------------
            bsb = big.tile([128, GRP * SEQ * 4], F32, tag="B")
            nc.vector.tensor_scalar(out=bsb[:], in0=xsb[:], scalar1=127.0,
                                    scalar2=None, op0=OP.is_gt)
            bv = bsb[:].rearrange("p (g t k) -> p g t k", g=GRP, k=4)
            bk = [bv[:, :, :, k] for k in range(4)]                  # each (128, g, t)

            q01 = ph1.tile([128, GRP * SEQ], F32, tag="q01")
            q23 = ph1.tile([128, GRP * SEQ], F32, tag="q23")
            gt = lambda tl: tl[:].rearrange("p (g t) -> p g t", g=GRP)
            nc.vector.tensor_tensor(out=gt(q01), in0=bk[0], in1=bk[1], op=OP.mult)
            nc.vector.tensor_tensor(out=gt(q23), in0=bk[2], in1=bk[3], op=OP.mult)
            bfl = bsb[:].rearrange("p (n k) -> p n k", k=4)
            rs = []
            for j in range(4):
                r = ph1.tile([128, GRP * SEQ], F32, tag=f"r{j}")
                nc.vector._custom_dve(
                    rs2, out=r[:], in0=bfl[:, :, 2], in1=bfl[:, :, 3],
                    s0=float(_CML[j, 0]), s1=float(_CML[j, 1]),
                    imm2=float(_CML[j, 2]))
                nc.vector.affine_then_add(out=gt(r), in0=gt(q23), in1=gt(r),
                                          scale=float(_CML[j, 3]), bias=0.0)
                rs.append(r)
            m = ph1.tile([128, GRP * SEQ], F32, tag="m")
            vcv = big.tile([128, GRP * SEQ], F32, tag="V")           # conv (g, t)
            nc.vector.tensor_tensor(out=gt(m), in0=bk[0], in1=gt(rs[1]), op=OP.mult)
            nc.vector.tensor_tensor(out=gt(vcv), in0=gt(rs[0]), in1=gt(m), op=OP.add)
            nc.vector.tensor_tensor(out=gt(m), in0=bk[1], in1=gt(rs[2]), op=OP.mult)
            nc.vector.tensor_tensor(out=gt(vcv), in0=gt(vcv), in1=gt(m), op=OP.add)
            nc.vector.tensor_tensor(out=gt(m), in0=gt(q01), in1=gt(rs[3]), op=OP.mult)
            nc.vector.tensor_tensor(out=gt(vcv), in0=gt(vcv), in1=gt(m), op=OP.add)

            # ---------------- phase 1b: populate hE state buffers ----------
            # hE_g block t (cols t*6 .. t*6+5) = [h0..h3, conv_t, 1.0]
            hE = [big.tile([128, (SEQ + 1) * 6], F32, tag=f"HE{g}", name=f"HE{g}") for g in range(GRP)]
            CS = [big.tile([128, 97], F32, tag=f"CS{g}", name=f"CS{g}") for g in range(GRP)]
            Z = [big.tile([128, 4 * UBLK], F32, tag=f"Z{g}", name=f"Z{g}") for g in range(GRP)]
            STA = [big.tile([128, 8], F32, tag=f"STA{g}", name=f"STA{g}") for g in range(GRP)]
            STB = [big.tile([128, 12], F32, tag=f"STB{g}", name=f"STB{g}") for g in range(GRP)]
            STC = [big.tile([128, 8], F32, tag=f"STC{g}", name=f"STC{g}") for g in range(GRP)]

            for g in range(GRP):
                hv = hE[g][:].rearrange("p (t k) -> p t k", k=6)
                # conv slots: hE[t*6+4] = vcv[g, t]
                nc.vector.tensor_scalar(
                    out=hv[:, 0:SEQ, 4], in0=vcv[:].rearrange(
                        "p (g t) -> p g t", g=GRP)[:, g, :],
                    scalar1=1.0, scalar2=None, op0=OP.mult)
                nc.vector.memset(hv[:, 0:SEQ, 5], 1.0)
                nc.vector.memset(hv[:, 0, 0:4], 0.0)
                nc.vector.memset(STA[g][:, 0:4], 0.0)      # s2 = 0
                nc.vector.memset(CS[g][:, 0:1], 0.0)       # guard column

            # ---------------- phase 2: the scan ----------------
            zv = [Z[g][:].rearrange("p (a u) -> p a u", u=UBLK) for g in range(GRP)]
            for t in range(SEQ):
                # emit in ABAB (group-interleaved) order per stage so every
                # dependency sits at instruction distance >= 2
                for g in range(GRP):  # M6
                    hblk = hE[g][:, t * 6:(t + 1) * 6].unsqueeze(1)
                    nc.vector._custom_dve(
                        mulscan,
                        out=CS[g][:, 1:97],
                        in0=hblk.broadcast_to((128, 16, 6)),
                        in1=wxsb[:, 0:96])
                for g in range(GRP):  # DW: th = wrap(ends - starts) -> th slots
                    ends = CS[g][:, 1:97].rearrange("p (m k) -> p m k", k=6)[:, :, 5]
                    starts = CS[g][:, 0:96].rearrange("p (m k) -> p m k", k=6)[:, :, 0]
                    nc.vector._custom_dve(
                        diffwrap,
                        out=zv[g][:, :, 28:32],
                        in0=ends, in1=starts,
                        s0=float(1.0 / _SLAM), s1=PI, imm2=2 * PI)
                for g in range(GRP):  # SIN: z_w at slots 0,6,12,18
                    nc.vector._custom_dve(
                        oddp7,
                        out=zv[g][:, :, 0:24].rearrange("p a (w r) -> p a w r", r=6)[:, :, :, 0],
                        in0=zv[g][:, :, 28:32],
                        s0=_SIN7S[0], s1=_SIN7S[1], imm2=_SIN7S[2])
                for g in range(GRP):  # P1: [q1,m12,b23] = z[0,1,2]*z[1,2,3]
                    nc.vector.tensor_tensor(
                        out=zv[g][:, :, 24:27],
                        in0=zv[g][:, :, 0:18].rearrange("p a (w r) -> p a w r", r=6)[:, :, :, 0],
                        in1=zv[g][:, :, 6:24].rearrange("p a (w r) -> p a w r", r=6)[:, :, :, 0],
                        op=OP.mult)
                for g in range(GRP):  # P2: [q2,q0,q3] = [z2,z3,q1]*[q1,m12,b23]
                    nc.vector.tensor_tensor(
                        out=zv[g][:, :, 21:24],
                        in0=zv[g][:, :, 12:30].rearrange("p a (w r) -> p a w r", r=6)[:, :, :, 0],
                        in1=zv[g][:, :, 24:27],
                        op=OP.mult)
                for g in range(GRP):  # TS: u_a = 1 + tanh(q/2), a in {f,i,o}
                    # in0 = contiguous [q2,q0,q3,q1] window; PERM order
                    nc.vector._custom_dve(
                        tanh5p,
                        out=STB[g][:, 0:12].rearrange("p (a w) -> p a w", w=4),
                        in0=zv[g][:, 0:3, 21:25],
                        s0=tgh[2], s1=tgh[1], imm2=tgh[0])
                for g in range(GRP):  # TU: t_u = tanh(q), a = u
                    nc.vector._custom_dve(
                        tanh5,
                        out=STA[g][:, 4:8],
                        in0=zv[g][:, 3, 21:25],
                        s0=tg[2], s1=tg[1], imm2=tg[0])
                for g in range(GRP):  # AB: [A,B] = [u_f,u_i] * [s2,tu]
                    nc.vector.tensor_tensor(
                        out=STC[g][:].rearrange("p (j w) -> p j w", w=4),
                        in0=STB[g][:, 0:8].rearrange("p (j w) -> p j w", w=4),
                        in1=STA[g][:].rearrange("p (j w) -> p j w", w=4),
                        op=OP.mult)
                for g in range(GRP):  # S2: s2' = 0.5*A + B
                    nc.vector.scalar_tensor_tensor(
                        out=STA[g][:, 0:4], in0=STC[g][:, 0:4], scalar=0.5,
                        in1=STC[g][:, 4:8], op0=OP.mult, op1=OP.add)
                for g in range(GRP):  # HF: h' = u_o * tanh5(s2'/2) -> hE[t+1]
                    nc.vector._custom_dve(
                        hfuse,
                        out=hE[g][:, (t + 1) * 6:(t + 1) * 6 + 4],
                        in0=STA[g][:, 0:4],
                        in1=STB[g][:, 8:12],
                        s0=tch[2], s1=tch[1], imm2=tch[0])

            # ---------------- phase 3: y = h @ (W_out/2) + b_out ----------
            # cumsum of h*wo along (t, w), then segment sums via guard diffs
            YC = [big.tile([128, SEQ * 4 + 1], F32, tag=f"YC{g}", name=f"YC{g}")
                  for g in range(GRP)]
            yy = big.tile([128, GRP * SEQ], F32, tag="Y")
            for g in range(GRP):
                nc.vector.memset(YC[g][:, 0:1], 0.0)
            for g in range(GRP):
                hsv = (hE[g][:, 6:].rearrange("p (t k) -> p t k", k=6)[:, :, 0:4])
                wo = (cpsb[:, 4:8].unsqueeze(1).broadcast_to((128, SEQ, 4)))
                nc.vector._custom_dve(
                    mulscan,
                    out=YC[g][:, 1:SEQ * 4 + 1],
                    in0=hsv, in1=wo)
            for g in range(GRP):
                yends = YC[g][:, 1:SEQ * 4 + 1].rearrange(
                    "p (m k) -> p m k", k=4)[:, :, 3]
                ystarts = YC[g][:, 0:SEQ * 4].rearrange(
                    "p (m k) -> p m k", k=4)[:, :, 0]
                nc.vector._custom_dve(
                    diffadd,
                    out=yy[:].rearrange("p (g t) -> p g t", g=GRP)[:, g],
                    in0=yends, in1=ystarts,
                    s0=cpsb[:, 8:9])
            nc.sync.dma_start(
                y_d.rearrange("(g p) t -> p g t", p=128),
                yy[:].rearrange("p (g t) -> p g t", g=GRP),
            )
            if debug:
                for g in range(GRP):
                    nc.sync.dma_start(
                        dh_d[:].rearrange("p (g n) -> p g n", g=GRP)[:, g],
                        hE[g][:])

    nc.compile()
    return nc


def _pack_consts(W_f, b_f, W_i, b_i, W_u, b_u, W_o, b_o,
                 rx_f, rx_i, rx_u, rx_o, W_out, b_out):
    """wx[128,96] (m=(a,w), k6=[0.5*Wh permuted, Wx, beta']) and cp[128,16].

    hE h-slots hold h in PERM wire order, so whx's Wh columns are permuted
    to match: whx[m, j] = 0.5*W[a][w_m, 1 + PERM[j]]."""
    Ws = [W_f, W_i, W_o, W_u]          # gate order f,i,o,u
    bs = [b_f, b_i, b_o, b_u]
    rxs = [rx_f, rx_i, rx_o, rx_u]
    whx = np.zeros((4, 4, 6), np.float32)
    for a in range(4):
        Wa = np.asarray(Ws[a], np.float32)
        whx[a, :, 0:4] = 0.5 * Wa[:, 1:5][:, PERM]
        whx[a, :, 4] = Wa[:, 0]
        beta = (np.asarray(bs[a], np.float32)
                + np.asarray(rxs[a], np.float32) + np.float32(np.pi / 2))
        whx[a, :, 5] = (beta + np.pi) % (2 * np.pi) - np.pi
    wx = np.tile(whx.reshape(1, 96), (128, 1)).astype(np.float32)

    cprow = np.zeros(16, np.float32)
    cprow[0] = float(_SIN7[0])                        # sin c3 latch const
    cprow[4:8] = 0.5 * np.asarray(W_out, np.float32)[0][PERM]
    cprow[8] = float(np.asarray(b_out, np.float32)[0])
    cp = np.tile(cprow[None], (128, 1)).astype(np.float32)

    # range check: theta must stay within [-3pi, 3pi] for the single wrap
    whabs = np.abs(whx[:, :, 0:4]).sum(axis=2) * 2.0
    bound = (np.abs(whx[:, :, 5]) + np.abs(whx[:, :, 4]) + whabs).max()
    assert bound < 3 * PI - 0.2, f"theta range {bound} too large for single wrap"
    return wx, cp


def kernel(**inputs):
    from concourse.bass_utils import run_bass_kernel_spmd

    x = np.ascontiguousarray(np.asarray(inputs["x"], np.float32)).reshape(BATCH, SEQ, 4)
    wx, cp = _pack_consts(**{k: v for k, v in inputs.items() if k != "x"})

    if "nc" not in _CACHE:
        _CACHE["nc"] = _build_program()
    nc = _CACHE["nc"]

    in_maps = []
    for cid in range(N_CORES):
        xs = np.ascontiguousarray(
            x[cid * BPC:(cid + 1) * BPC].reshape(BPC, SEQ * 4))
        in_maps.append({"xs": xs, "wx": wx, "cp": cp})

    res = run_bass_kernel_spmd(nc, in_maps, core_ids=list(range(N_CORES)))
    ys = [res.results[cid]["y"] for cid in range(N_CORES)]  # each (BPC, SEQ)
    full = np.concatenate(ys, axis=0)                       # (BATCH, SEQ)
    return np.ascontiguousarray(full.T)[:, :, None].astype(np.float32)


# revision 14
# speedup vs baseline: 1.0276x; 1.0001x over previous
"""Trainium2 Bass kernel for the quantum ConvLSTM reference.

Math reduction (validated to ~7e-3 rel vs the jax reference in numpy sim):
  * quantum_conv(patch) == T16[b] from thresholding the 4 pixels at 127;
    evaluated on-chip as a multilinear polynomial in the 4 bits.
  * qlayer(x, p) == [z1*z2*z3, z0*z1, z0*z1*z2, z0*z1*z2*z3] with
    z_w = cos(x_w + p_w); the LSTM scan becomes a small classical
    recurrence: theta = Wh@h + Wx*conv + beta; z = sin(theta + pi/2);
    gate products; sigmoid via (1+tanh(q/2))/2; c/h updates with the
    2c/2h scaling folded into constants.

Per-step pipeline (10 DVE ops per batch group, 2 groups interleaved
ABAB so every dependency sits at instruction distance >= 2 and the
vector engine's FIFO streams without semaphore stalls):
  M6   one multiply-cumsum of [h0..h3, conv_t, 1] x whx6 (96 els)
       -> full gate preactivation sums via guard-column differences
  DW   custom diff+wrap: wrap(CS[6m+6]-CS[6m]) into [-pi, pi]
  SIN  deg-7 odd sin (8 ALU stages, 4th coef via C3->Src1 latch)
  P1   strided multiply -> [q1, m12, b23] per gate unit
  P2   strided multiply -> [q2, q0, q3] per gate unit
  TS   1 + tanh(q/2) for sigmoid gates (deg-5, 0.5 folded into coefs)
  TU   tanh(q) for the u gate (deg-5)
  AB   paired multiply [u_f*s2, u_i*t_u]
  S2   s2' = 0.5*A + B (stock STT)
  HF   h' = u_o * tanh5(s2'/2), written into the next hE state block

Sharding: pure data parallel over batch (2048 -> 8 cores x 256 rows).
Each core: 128 partitions x 2 column-groups; seq scan of 512 steps fully
unrolled on-chip.
"""

import sys

import numpy as np

sys.path.insert(0, "/opt/trn_rl_repo")

N_CORES = 8
BATCH = 2048
SEQ = 512
BPC = BATCH // N_CORES          # 256 batch rows per core
GRP = BPC // 128                # 2 column groups of 128 partitions
PI = float(np.pi)

# Z-tile slot layout per gate unit (stride UBLK per unit a):
#   z0@0 z1@6 z2@12 z3@18   (sin output, stride 6)
#   q2@21 q0@22 q3@23       (P2 output, stride 1)
#   q1@24 m12@25 b23@26     (P1 output, stride 1)
#   th@28..31               (DW output)
# The tanh stage reads the contiguous window [21..25) = [q2,q0,q3,q1],
# i.e. wire order PERM = [2,0,3,1]. All per-wire state (u, tu, s2, A, B,
# h-slots in hE) is stored in this permuted order; whx's Wh columns and
# W_out are permuted host-side to match, so every AP is a plain
# positive-stride window.
UBLK = 32
PERM = [2, 0, 3, 1]

# STATE-tile slot layout per group:
#   s2@0..3  tu@4..7  u_f@8..11 u_i@12..15 u_o@16..19  A@20..23 B@24..27
SBLK = 32

# ---------------------------------------------------------------------------
# Host-side constants: T16 lookup table + multilinear coefficients.
# ---------------------------------------------------------------------------
_RY_ANGLES = np.random.RandomState(0).uniform(0.0, 2.0 * np.pi, size=(2, 4)).astype(np.float32)


def _build_t16() -> np.ndarray:
    s = np.zeros((16, 2, 2, 2, 2), np.complex64)
    for b in range(16):
        bits = [(b >> 3) & 1, (b >> 2) & 1, (b >> 1) & 1, b & 1]
        s[(b, *bits)] = 1.0

    def ry(state, th, w):
        a0 = np.take(state, 0, axis=1 + w)
        a1 = np.take(state, 1, axis=1 + w)
        c = np.complex64(np.cos(np.float32(th) / 2))
        sn = np.complex64(np.sin(np.float32(th) / 2))
        return np.stack([c * a0 - sn * a1, sn * a0 + c * a1], axis=1 + w)

    def cnot(state, ctl, tgt):
        s0 = np.take(state, 0, axis=1 + ctl)
        s1 = np.take(state, 1, axis=1 + ctl)
        t_ax = 1 + tgt if tgt < ctl else tgt
        s1 = np.flip(s1, axis=t_ax)
        return np.stack([s0, s1], axis=1 + ctl)

    for layer in range(2):
        for w in range(4):
            s = ry(s, _RY_ANGLES[layer, w], w)
        for w in range(3):
            s = cnot(s, w, w + 1)
    probs = np.abs(s) ** 2
    cols = []
    for w in range(4):
        other = tuple(a for a in range(1, 5) if a != 1 + w)
        cols.append(probs.sum(axis=other)[:, 1])
    return np.stack(cols, axis=1).mean(axis=1)  # (16,)


def _multilinear_coeffs(t16: np.ndarray) -> np.ndarray:
    """C[4][4] with T16[b] = sum_jk C[j,k]*u_j*v_k, u=[1,b0,b1,b0b1], v=[1,b2,b3,b2b3]."""
    m = np.zeros((16, 16))
    for b in range(16):
        b0, b1, b2, b3 = (b >> 3) & 1, (b >> 2) & 1, (b >> 1) & 1, b & 1
        u = [1, b0, b1, b0 * b1]
        v = [1, b2, b3, b2 * b3]
        for j in range(4):
            for k in range(4):
                m[b, j * 4 + k] = u[j] * v[k]
    return np.linalg.solve(m, t16.astype(np.float64)).reshape(4, 4)


_T16 = _build_t16()
_CML = _multilinear_coeffs(_T16)


def _fit_odd(f, hi, ncoef):
    """Near-minimax odd fit f(x) ~ x*p(x^2) on [-hi, hi]; p coeffs ascending."""
    n = 4000
    k = np.arange(n)
    x = np.cos(np.pi * (k + 0.5) / n) * hi
    y = x * x
    a = np.stack([x * y ** j for j in range(ncoef)], axis=1)
    tgt = f(x)
    c, *_ = np.linalg.lstsq(a, tgt, rcond=None)
    for _ in range(60):
        r = a @ c - tgt
        w = (np.abs(r) + 1e-12) ** 0.5
        c, *_ = np.linalg.lstsq(a * w[:, None], tgt * w, rcond=None)
    return c


_SIN7 = _fit_odd(np.sin, np.pi, 4)       # deg-7 odd sin on [-pi, pi], ~2.6e-4
# Pre-scale x by 1/lambda so the leading coefficient becomes exactly 1.0 and
# the sin op needs only 3 constants (no C3 latch): sin(x) ~ x'*(c0'+c1'y'+c2'y'^2+y'^3)
_SLAM = -np.abs(1.0 / _SIN7[3]) ** (1 / 7) if _SIN7[3] < 0 else (1.0 / _SIN7[3]) ** (1 / 7)
_SIN7S = [float(_SIN7[k] * _SLAM ** (2 * k + 1)) for k in range(4)]
_TG5 = _fit_odd(np.tanh, 1.0, 3)         # deg-5 tanh on [-1, 1], ~4e-4
_TC5 = _fit_odd(np.tanh, 2.0, 3)         # deg-5 tanh on [-2, 2] for tanh(c)
# tanh(q/2) coefs: c_k' = c_k / 2^(2k+1)
_TG5H = _TG5 * np.array([1 / 2 ** (2 * k + 1) for k in range(3)])
# tanh(s2/2) coefs for HFUSE (s2 = 2c)
_TC5H = _TC5 * np.array([1 / 2 ** (2 * k + 1) for k in range(3)])

_CACHE = {}


def _register_custom_ops():
    """Register fused DVE ops (idempotent). Shas are pinned by bootstrap:
    compile once with an empty pin, parse the actual sha from the error."""
    import re
    import concourse.dve_ops as dve_ops_mod
    from concourse.dve_ops import OPS, DveOp
    from concourse.dve_spec import (
        Spec, Src0, Src1, C0, C1, C2, Zero, One, Latch, scan, AluOp, sq,
    )

    have = {o.name for o in OPS}

    def make(name, spec):
        if name in have:
            return next(o for o in OPS if o.name == name)
        probe = DveOp(name, spec, subdim=False, uops_sha={})
        OPS.append(probe)
        dve_ops_mod._SUB_OPCODE_FOR_NAME[name] = (
            dve_ops_mod._CUSTOM_DVE_ROW_BASE + len(OPS) - 1)
        shas = {}
        for ver in ("v3", "v4"):
            try:
                probe.compile(ver)
            except ValueError as e:
                mm = re.search(r"(\b[0-9a-f]{16})\b", str(e))
                shas[ver] = mm.group(1)
        OPS.remove(probe)
        op = DveOp(name, spec, subdim=False, uops_sha=shas)
        OPS.append(op)
        return op

    # out = cumsum(in0 * in1) along the free stream (same as baseline)
    def _ref_mulscan(in0, in1, c0, c1, c2):
        p = (in0.astype(np.float32) * in1).reshape(in0.shape[0], -1)
        return np.cumsum(p, axis=1, dtype=np.float32).reshape(in0.shape)

    mulscan = make("MULSCAN_ANT", Spec(
        body=scan(AluOp.ADD, Src0 * Src1),
        reference=_ref_mulscan))

    # out = c0 * wrap(in0 - in1) into [-c1, c1] by one period c2
    d = Src0 - Src1
    dws = (d + C2 * ((d < (Zero - C1)) - (C1 < d))) * C0
    def _ref_dws(in0, in1, c0, c1, c2):
        dd = (in0.astype(np.float32) - in1).astype(np.float32)
        return (c0 * (dd + c2 * ((dd < -c1).astype(np.float32)
                                 - (c1 < dd).astype(np.float32)))).astype(np.float32)
    diffwrap = make("DIFFWRAPS_ANT", Spec(body=dws, reference=_ref_dws))

    # out = in0 - in1 + c0  (phase-3 segment-sum extraction + bias)
    da = Src0 - Src1 + C0
    def _ref_da(in0, in1, c0, c1, c2):
        return (in0.astype(np.float32) - in1 + c0).astype(np.float32)
    diffadd = make("DIFFADD_ANT", Spec(body=da, reference=_ref_da))

    # out = c0 + c1*in0 + c2*in1  (phase-1 multilinear partial)
    rs2b = C0 + C1 * Src0 + C2 * Src1
    def _ref_rs2(in0, in1, c0, c1, c2):
        return (c0 + c1 * in0.astype(np.float32) + c2 * in1).astype(np.float32)
    rs2 = make("RS2_ANT", Spec(body=rs2b, reference=_ref_rs2))

    # out = x*(((y + c2)*y + c1)*y + c0), y = x^2 (deg-7 odd, monic leading)
    y7 = sq(Src0)
    p7 = (((y7 + C2) * y7 + C1) * y7 + C0) * Src0
    def _ref_p7(in0, in1, c0, c1, c2):
        yy = in0.astype(np.float32) ** 2
        return ((((yy + c2) * yy + c1) * yy + c0) * in0).astype(np.float32)
    oddp7 = make("ODDP7N_ANT", Spec(body=p7, reference=_ref_p7))

    # out = ((c0*y + c1)*y + c2)*x + 1, y = x^2  (deg-5 odd + 1)
    y5 = sq(Src0)
    t5p = ((C0 * y5 + C1) * y5 + C2) * Src0 + One
    def _ref_t5p(in0, in1, c0, c1, c2):
        yy = in0.astype(np.float32) ** 2
        return (((c0 * yy + c1) * yy + c2) * in0 + 1.0).astype(np.float32)
    tanh5p = make("TANH5P_ANT", Spec(body=t5p, reference=_ref_t5p))

    # out = ((c0*y + c1)*y + c2)*x, y = x^2  (deg-5 odd)
    t5 = ((C0 * y5 + C1) * y5 + C2) * Src0
    def _ref_t5(in0, in1, c0, c1, c2):
        yy = in0.astype(np.float32) ** 2
        return (((c0 * yy + c1) * yy + c2) * in0).astype(np.float32)
    tanh5 = make("TANH5_ANT", Spec(body=t5, reference=_ref_t5))

    # out = in1 * ((c0*y + c1)*y + c2)*in0, y = in0^2  (h update)
    hf = Src1 * (((C0 * y5 + C1) * y5 + C2) * Src0)
    def _ref_hf(in0, in1, c0, c1, c2):
        yy = in0.astype(np.float32) ** 2
        return (in1 * (((c0 * yy + c1) * yy + c2) * in0)).astype(np.float32)
    hfuse = make("HFUSE_ANT", Spec(body=hf, reference=_ref_hf))

    return mulscan, diffwrap, diffadd, rs2, oddp7, tanh5p, tanh5, hfuse


def _build_program(debug=False):
    """Build + compile the (weights-independent) single-core SPMD Bass program."""
    import concourse.bass as bass
    import concourse.mybir as mybir
    import concourse.tile as tile
    from concourse import bacc

    F32 = mybir.dt.float32
    OP = mybir.AluOpType

    mulscan, diffwrap, diffadd, rs2, oddp7, tanh5p, tanh5, hfuse = _register_custom_ops()

    nc = bacc.Bacc(None, target_bir_lowering=False)

    x_d = nc.dram_tensor("xs", [BPC, SEQ * 4], F32, kind="ExternalInput")
    wx_d = nc.dram_tensor("wx", [128, 96], F32, kind="ExternalInput")
    cp_d = nc.dram_tensor("cp", [128, 16], F32, kind="ExternalInput")
    y_d = nc.dram_tensor("y", [BPC, SEQ], F32, kind="ExternalOutput")
    if debug:
        dh_d = nc.dram_tensor("dbg_h", [128, GRP * (SEQ + 1) * 6], F32, kind="ExternalOutput")

    sc7 = [float(v) for v in _SIN7]
    tgh = [float(v) for v in _TG5H]
    tg = [float(v) for v in _TG5]
    tch = [float(v) for v in _TC5H]

    with tile.TileContext(nc) as tc:
        with (
            tc.tile_pool(name="big", bufs=1) as big,
            tc.tile_pool(name="ph1", bufs=1) as ph1,
        ):
            # ---------------- load ----------------
            xsb = big.tile([128, GRP * SEQ * 4], F32, tag="X")       # (g, t, k)
            nc.sync.dma_start(
                xsb[:].rearrange("p (g n) -> p g n", g=GRP),
                x_d.rearrange("(g p) n -> p g n", p=128),
            )
            wxsb = big.tile([128, 96], F32, tag="WX")                # (a, w, k6)
            nc.sync.dma_start(wxsb[:], wx_d[:])
            cpsb = big.tile([128, 16], F32, tag="CP")
            nc.sync.dma_start(cpsb[:], cp_d[:])

            # ---------------- phase 1: bits -> conv ----------------
            bsb = big.tile([128, GRP * SEQ * 4], F32, tag="B")
            nc.vector.tensor_scalar(out=bsb[:], in0=xsb[:], scalar1=127.0,
                                    scalar2=None, op0=OP.is_gt)
            bv = bsb[:].rearrange("p (g t k) -> p g t k", g=GRP, k=4)
            bk = [bv[:, :, :, k] for k in range(4)]                  # each (128, g, t)

            q01 = ph1.tile([128, GRP * SEQ], F32, tag="q01")
            q23 = ph1.tile([128, GRP * SEQ], F32, tag="q23")
            gt = lambda tl: tl[:].rearrange("p (g t) -> p g t", g=GRP)
            nc.vector.tensor_tensor(out=gt(q01), in0=bk[0], in1=bk[1], op=OP.mult)
            nc.vector.tensor_tensor(out=gt(q23), in0=bk[2], in1=bk[3], op=OP.mult)
            bfl = bsb[:].rearrange("p (n k) -> p n k", k=4)
            rs = []
            for j in range(4):
                r = ph1.tile([128, GRP * SEQ], F32, tag=f"r{j}")
                nc.vector._custom_dve(
                    rs2, out=r[:], in0=bfl[:, :, 2], in1=bfl[:, :, 3],
                    s0=float(_CML[j, 0]), s1=float(_CML[j, 1]),
                    imm2=float(_CML[j, 2]))
                nc.vector.affine_then_add(out=gt(r), in0=gt(q23), in1=gt(r),
                                          scale=float(_CML[j, 3]), bias=0.0)
                rs.append(r)
            m = ph1.tile([128, GRP * SEQ], F32, tag="m")
            vcv = big.tile([128, GRP * SEQ], F32, tag="V")           # conv (g, t)
            nc.vector.tensor_tensor(out=gt(m), in0=bk[0], in1=gt(rs[1]), op=OP.mult)
            nc.vector.tensor_tensor(out=gt(vcv), in0=gt(rs[0]), in1=gt(m), op=OP.add)
            nc.vector.tensor_tensor(out=gt(m), in0=bk[1], in1=gt(rs[2]), op=OP.mult)
            nc.vector.tensor_tensor(out=gt(vcv), in0=gt(vcv), in1=gt(m), op=OP.add)
            nc.vector.tensor_tensor(out=gt(m), in0=gt(q01), in1=gt(rs[3]), op=OP.mult)
            nc.vector.tensor_tensor(out=gt(vcv), in0=gt(vcv), in1=gt(m), op=OP.add)

            # ---------------- phase 1b: populate hE state buffers ----------
            # hE_g block t (cols t*6 .. t*6+5) = [h0..h3, conv_t, 1.0]
            hE = [big.tile([128, (SEQ + 1) * 6], F32, tag=f"HE{g}", name=f"HE{g}") for g in range(GRP)]
            CS = [big.tile([128, 97], F32, tag=f"CS{g}", name=f"CS{g}") for g in range(GRP)]
            Z = [big.tile([128, 4 * UBLK], F32, tag=f"Z{g}", name=f"Z{g}") for g in range(GRP)]
            STA = [big.tile([128, 8], F32, tag=f"STA{g}", name=f"STA{g}") for g in range(GRP)]
            STB = [big.tile([128, 12], F32, tag=f"STB{g}", name=f"STB{g}") for g in range(GRP)]
            STC = [big.tile([128, 8], F32, tag=f"STC{g}", name=f"STC{g}") for g in range(GRP)]

            for g in range(GRP):
                hv = hE[g][:].rearrange("p (t k) -> p t k", k=6)
                # conv slots: hE[t*6+4] = vcv[g, t]
                nc.vector.tensor_scalar(
                    out=hv[:, 0:SEQ, 4], in0=vcv[:].rearrange(
                        "p (g t) -> p g t", g=GRP)[:, g, :],
                    scalar1=1.0, scalar2=None, op0=OP.mult)
                nc.vector.memset(hv[:, 0:SEQ, 5], 1.0)
                nc.vector.memset(hv[:, 0, 0:4], 0.0)
                nc.vector.memset(STA[g][:, 0:4], 0.0)      # s2 = 0
                nc.vector.memset(CS[g][:, 0:1], 0.0)       # guard column

            # ---------------- phase 2: the scan ----------------
            zv = [Z[g][:].rearrange("p (a u) -> p a u", u=UBLK) for g in range(GRP)]
            for t in range(SEQ):
                # emit in ABAB (group-interleaved) order per stage so every
                # dependency sits at instruction distance >= 2
                for g in range(GRP):  # M6
                    hblk = hE[g][:, t * 6:(t + 1) * 6].unsqueeze(1)
                    nc.vector._custom_dve(
                        mulscan,
                        out=CS[g][:, 1:97],
                        in0=hblk.broadcast_to((128, 16, 6)),
                        in1=wxsb[:, 0:96])
                for g in range(GRP):  # DW: th = wrap(ends - starts) -> th slots
                    ends = CS[g][:, 1:97].rearrange("p (m k) -> p m k", k=6)[:, :, 5]
                    starts = CS[g][:, 0:96].rearrange("p (m k) -> p m k", k=6)[:, :, 0]
                    nc.vector._custom_dve(
                        diffwrap,
                        out=zv[g][:, :, 28:32],
                        in0=ends, in1=starts,
                        s0=float(1.0 / _SLAM), s1=PI, imm2=2 * PI)
                for g in range(GRP):  # SIN: z_w at slots 0,6,12,18
                    nc.vector._custom_dve(
                        oddp7,
                        out=zv[g][:, :, 0:24].rearrange("p a (w r) -> p a w r", r=6)[:, :, :, 0],
                        in0=zv[g][:, :, 28:32],
                        s0=_SIN7S[0], s1=_SIN7S[1], imm2=_SIN7S[2])
                for g in range(GRP):  # P1: [q1,m12,b23] = z[0,1,2]*z[1,2,3]
                    nc.vector.tensor_tensor(
                        out=zv[g][:, :, 24:27],
                        in0=zv[g][:, :, 0:18].rearrange("p a (w r) -> p a w r", r=6)[:, :, :, 0],
                        in1=zv[g][:, :, 6:24].rearrange("p a (w r) -> p a w r", r=6)[:, :, :, 0],
                        op=OP.mult)
                for g in range(GRP):  # P2: [q2,q0,q3] = [z2,z3,q1]*[q1,m12,b23]
                    nc.vector.tensor_tensor(
                        out=zv[g][:, :, 21:24],
                        in0=zv[g][:, :, 12:30].rearrange("p a (w r) -> p a w r", r=6)[:, :, :, 0],
                        in1=zv[g][:, :, 24:27],
                        op=OP.mult)
                for g in range(GRP):  # TS: u_a = 1 + tanh(q/2), a in {f,i,o}
                    # in0 = contiguous [q2,q0,q3,q1] window; PERM order
                    nc.vector._custom_dve(
                        tanh5p,
                        out=STB[g][:, 0:12].rearrange("p (a w) -> p a w", w=4),
                        in0=zv[g][:, 0:3, 21:25],
                        s0=tgh[2], s1=tgh[1], imm2=tgh[0])
                for g in range(GRP):  # TU: t_u = tanh(q), a = u
                    nc.vector._custom_dve(
                        tanh5,
                        out=STA[g][:, 4:8],
                        in0=zv[g][:, 3, 21:25],
                        s0=tg[2], s1=tg[1], imm2=tg[0])
                for g in range(GRP):  # AB: [A,B] = [u_f,u_i] * [s2,tu]
                    nc.vector.tensor_tensor(
                        out=STC[g][:].rearrange("p (j w) -> p j w", w=4),
                        in0=STB[g][:, 0:8].rearrange("p (j w) -> p j w", w=4),
                        in1=STA[g][:].rearrange("p (j w) -> p j w", w=4),
                        op=OP.mult)
                for g in range(GRP):  # S2: s2' = 0.5*A + B
                    nc.vector.scalar_tensor_tensor(
                        out=STA[g][:, 0:4], in0=STC[g][:, 0:4], scalar=0.5,
                        in1=STC[g][:, 4:8], op0=OP.mult, op1=OP.add)
                for g in range(GRP):  # HF: h' = u_o * tanh5(s2'/2) -> hE[t+1]
                    nc.vector._custom_dve(
                        hfuse,
                        out=hE[g][:, (t + 1) * 6:(t + 1) * 6 + 4],
                        in0=STA[g][:, 0:4],
                        in1=STB[g][:, 8:12],
                        s0=tch[2], s1=tch[1], imm2=tch[0])

            # ---------------- phase 3: y = h @ (W_out/2) + b_out ----------
            # cumsum of h*wo along (t, w), then segment sums via guard diffs
            YC = [big.tile([128, SEQ * 4 + 1], F32, tag=f"YC{g}", name=f"YC{g}")
                  for g in range(GRP)]
            yy = big.tile([128, GRP * SEQ], F32, tag="Y")
            for g in range(GRP):
                nc.vector.memset(YC[g][:, 0:1], 0.0)
            for g in range(GRP):
                hsv = (hE[g][:, 6:].rearrange("p (t k) -> p t k", k=6)[:, :, 0:4])
                wo = (cpsb[:, 4:8].unsqueeze(1).broadcast_to((128, SEQ, 4)))
                nc.vector._custom_dve(
                    mulscan,
                    out=YC[g][:, 1:SEQ * 4 + 1],
                    in0=hsv, in1=wo)
            for g in range(GRP):
                yends = YC[g][:, 1:SEQ * 4 + 1].rearrange(
                    "p (m k) -> p m k", k=4)[:, :, 3]
                ystarts = YC[g][:, 0:SEQ * 4].rearrange(
                    "p (m k) -> p m k", k=4)[:, :, 0]
                nc.vector._custom_dve(
                    diffadd,
                    out=yy[:].rearrange("p (g t) -> p g t", g=GRP)[:, g],
                    in0=yends, in1=ystarts,
                    s0=cpsb[:, 8:9])
            nc.sync.dma_start(
                y_d.rearrange("(g p) t -> p g t", p=128),
                yy[:].rearrange("p (g t) -> p g t", g=GRP),
            )
            if debug:
                for g in range(GRP):
                    nc.sync.dma_start(
                        dh_d[:].rearrange("p (g n) -> p g n", g=GRP)[:, g],
                        hE[g][:])

    nc.compile()
    return nc


def _pack_consts(W_f, b_f, W_i, b_i, W_u, b_u, W_o, b_o,
                 rx_f, rx_i, rx_u, rx_o, W_out, b_out):
    """wx[128,96] (m=(a,w), k6=[0.5*Wh permuted, Wx, beta']) and cp[128,16].

    hE h-slots hold h in PERM wire order, so whx's Wh columns are permuted
    to match: whx[m, j] = 0.5*W[a][w_m, 1 + PERM[j]]."""
    Ws = [W_f, W_i, W_o, W_u]          # gate order f,i,o,u
    bs = [b_f, b_i, b_o, b_u]
    rxs = [rx_f, rx_i, rx_o, rx_u]
    whx = np.zeros((4, 4, 6), np.float32)
    for a in range(4):
        Wa = np.asarray(Ws[a], np.float32)
        whx[a, :, 0:4] = 0.5 * Wa[:, 1:5][:, PERM]
        whx[a, :, 4] = Wa[:, 0]
        beta = (np.asarray(bs[a], np.float32)
                + np.asarray(rxs[a], np.float32) + np.float32(np.pi / 2))
        whx[a, :, 5] = (beta + np.pi) % (2 * np.pi) - np.pi
    wx = np.tile(whx.reshape(1, 96), (128, 1)).astype(np.float32)

    cprow = np.zeros(16, np.float32)
    cprow[0] = float(_SIN7[0])                        # sin c3 latch const
    cprow[4:8] = 0.5 * np.asarray(W_out, np.float32)[0][PERM]
    cprow[8] = float(np.asarray(b_out, np.float32)[0])
    cp = np.tile(cprow[None], (128, 1)).astype(np.float32)

    # range check: theta must stay within [-3pi, 3pi] for the single wrap
    whabs = np.abs(whx[:, :, 0:4]).sum(axis=2) * 2.0
    bound = (np.abs(whx[:, :, 5]) + np.abs(whx[:, :, 4]) + whabs).max()
    assert bound < 3 * PI - 0.2, f"theta range {bound} too large for single wrap"
    return wx, cp


def kernel(**inputs):
    from concourse.bass_utils import run_bass_kernel_spmd

    x = np.ascontiguousarray(np.asarray(inputs["x"], np.float32)).reshape(BATCH, SEQ, 4)
    wx, cp = _pack_consts(**{k: v for k, v in inputs.items() if k != "x"})

    if "nc" not in _CACHE:
        _CACHE["nc"] = _build_program()
    nc = _CACHE["nc"]

    in_maps = []
    for cid in range(N_CORES):
        xs = np.ascontiguousarray(
            x[cid * BPC:(cid + 1) * BPC].reshape(BPC, SEQ * 4))
        in_maps.append({"xs": xs, "wx": wx, "cp": cp})

    res = run_bass_kernel_spmd(nc, in_maps, core_ids=list(range(N_CORES)))
    ys = [res.results[cid]["y"] for cid in range(N_CORES)]  # each (BPC, SEQ)
    full = np.concatenate(ys, axis=0)                       # (BATCH, SEQ)
    return np.ascontiguousarray(full.T)[:, :, None].astype(np.float32)
